# revision 74
# baseline (speedup 1.0000x reference)
"""AttnGraphSAGE on 8 Trainium2 NeuronCores (Bass/Tile) — v2.

Math restructuring (unchanged from v1): attention logits depend only on the
SOURCE node, so the whole edge phase is ONE segment-sum over dst of per-src
rows G[n] = [E_0*x_jm_0 (64) | E_1*x_jm_1 (64) | E_0 | E_1] (130 values).

v2 performance changes (2268us -> ~1274us on 8 cores):
  * G rows are bf16, 256-elem / 512B strides (was f32 768B): halves the
    random-gather HBM traffic and the AllGather volume.  All matmul operands
    (weights, activations, indicator) are bf16 -> 1-pass PE instead of 4.
  * The G table is AllGather'd in FOUR per-core row regions; each region's
    collective is issued as soon as phase A finishes its rows, so early
    regions' gathers overlap both the rest of phase A and the later
    AllGathers.  Every region stays < 32768 total rows, so region base
    addresses double as the int16-index split.
  * Gather calls are PACKED to exactly 1024 indices (the Q7 per-call
    ceiling) spanning dst-block boundaries; each call fills 8 consecutive
    subtiles of a 32-subtile ring arena per region.  A block's indicator
    matmuls consume the subtiles it touches; boundary subtiles are consumed
    by both adjacent blocks with foreign slots killed by dstloc=-1.  This
    minimizes Q7 descriptor-generation calls (~7ns/idx on the critical
    engine) with zero padding waste.
  * Synchronous (immediate) gather calls: measured cheaper per idx than the
    PREPARE_ONLY+trigger path, and the 4-queue rotation overlaps the DMA
    with subsequent descriptor generation anyway.  (GNN_ASYNC=1 selects the
    prepare/trigger path, kept for experiments.)
  * PSUM->bf16 casts and the per-head 1/denom scaling run on the Scalar
    engine (per-partition scale operand), and the is_equal indicator build
    compares against a materialized iota operand, unloading the DVE which
    is co-critical with the Q7/DMA during the edge phase.
  * BN stats reduce in block-aligned halves (first half starts during the
    phase-B tail) and the BN ReLU applies per chunk so the next layer's
    matmuls start on early chunks.
  * Per-core counts padded only to the max across the 8 cores so the
    program stays SPMD-uniform; 0-padded (no trailing -1 indices).
"""
import os
import sys
import types
import hashlib
import contextlib

sys.path.insert(0, "/opt/trn_rl_repo")

import numpy as np
import ml_dtypes

import concourse.bass as bass
import concourse.bacc as bacc
import concourse.mybir as mybir
from concourse import tile

# ---------------------------------------------------------------- constants
N = 50000
E = 800000
IN = 128
F = 64
H = 2
N_CORES = 8
NC_N = N // N_CORES          # 6250 nodes per core
BLK = 128                    # dst nodes per block
ROW = 256                    # G row stride in bf16 elems (512B)
GVAL = 2 * F + H             # 130 used cols
CHUNK = 512                  # phase-A node chunk
# G-table AllGather regions (per-core row ranges).  4 regions so the first
# regions' gathers start while later regions are still being computed /
# AllGather'd; each region stays < 32768 total rows for int16 indices.
R_BOUNDS = [0, 1280, 2560, 3840, 5120, NC_N]
NREG = len(R_BOUNDS) - 1
R_SIZE = [R_BOUNDS[i + 1] - R_BOUNDS[i] for i in range(NREG)]
AG_CHUNK = [(R_BOUNDS[i + 1] + CHUNK - 1) // CHUNK - 1 for i in range(NREG)]
ASPLIT = 3072                # BN stats half split (block-aligned)
BSPLIT = NC_N - ASPLIT
CAP = int(os.environ.get("GNN_CAP", "1024"))   # idxs per gather call (HW max)
ARENA_S = 24                 # ring-arena subtiles per region (3 calls)
GATE_D = 4                   # calls in flight per queue (ring + sem-slot cap)
F32 = mybir.dt.float32
BF16 = mybir.dt.bfloat16
FP8 = mybir.dt.float8e4
I16 = mybir.dt.int16
AF = mybir.ActivationFunctionType
OP = mybir.AluOpType
BN_EPS = 1e-5
LEAKY = 0.2
BF = ml_dtypes.bfloat16


# ------------------------------------------------------- axon profile shim
def _install_hookshim():
    if "antenv.axon_hooks" in sys.modules:
        return
    mod = types.ModuleType("antenv.axon_hooks")
    _h = [None]
    mod.set_axon_ntff_profile_hook = lambda h: _h.__setitem__(0, h)
    mod.get_axon_ntff_profile_hook = lambda: _h[0]
    try:
        import antenv
        sys.modules["antenv.axon_hooks"] = mod
        antenv.axon_hooks = mod
        from trn_agent_boot.trn_boot import _ntff_profile_via_ctypes
        mod.set_axon_ntff_profile_hook(
            _ntff_profile_via_ctypes("/opt/axon/libaxon_pjrt.so")
        )
    except Exception:
        pass


def remap_dmasw_waits(nc):
    """Remap waits on Tile's DMASW lane semaphores to the per-queue gather
    DMA-completion sems.

    Tile assigned each PREPARE_ONLY gather prep a DMASW lane (round-robin)
    and derived all downstream waits (consumers, ring flow control) as
    ``DMASW{lane} >= 16*tick``.  But the sem actually baked into the
    descriptors (and bumped by the SDMA engines) is our per-queue gdma sem,
    so those lane sems never move.  Each prep records its assigned
    (lane proc, tick); since each queue's ring is FIFO, the k-th prep of
    queue q has completed exactly when gdma{q} >= 16*k.  Rewrite every
    DMASW wait for (lane, tick) into the equivalent (and race-free)
    per-queue wait."""
    from concourse.tile_sem_assignment import PROC_NAME_TO_IDX
    inv_proc = {v: k for k, v in PROC_NAME_TO_IDX.items()}

    insts = []
    for func in nc.m.functions:
        for block in func.blocks:
            insts.extend(block.instructions)

    # (lane_name, 16*tick) -> (gdma sem id, gdma name, block-level target)
    lane_map = {}
    for inst in insts:
        if type(inst).__name__ == "InstDMAGatherAnt" and \
                getattr(inst, "gen_mode", 0) == 1:
            lane = inv_proc[inst.bass_scheduled_proc]
            upd = inst.sync_info.on_update[0]
            assert upd.ant_name.startswith("gdma"), upd.ant_name
            key = (lane, 16 * inst.bass_scheduled_tick)
            assert key not in lane_map, key
            lane_map[key] = (upd.id, upd.ant_name,
                             nc._gnn_prep_targets[inst.name])

    # waits with these prefixes are deferred from a prep to its trigger:
    # the prep only writes ring descriptors; the DMA (which actually touches
    # the arena / g_full) fires at the trigger, so enforcing reader-WAR and
    # collective deps there frees desc-gen to run ahead.
    XFER = ("PE_", "DVE_", "Act", "Collectives_")
    n = 0
    n_del = 0
    n_xfer = 0
    for func in nc.m.functions:
        for block in func.blocks:
            kept = []
            for inst in block.instructions:
                # Tile's per-prep DMASW shadow-sem maintenance is dead weight
                # once nothing uses the lane sems (1.65us of Pool each, plus
                # serializing ring-drain waits); the ring-capacity gates keep
                # the ring below capacity without it.
                if type(inst).__name__ == "InstIncSwdgeSem":
                    n_del += 1
                    continue
                kept.append(inst)
                si = inst.sync_info
                if not (si and si.on_wait):
                    continue
                changed = False
                new_waits = []
                trig = nc._gnn_prep_trig.get(inst.name)
                for w in si.on_wait:
                    if w.ant_name and w.ant_name.startswith("DMASW"):
                        lane = w.ant_name.split("_")[0]
                        sid, sname, thresh = lane_map[(lane, w.wait_value)]
                        new_waits.append(mybir.SyncWait(
                            sync_type="semaphore", id=sid,
                            wait_mode="sem-ge-imm",
                            wait_value=thresh, ant_name=sname))
                        changed = True
                    elif trig is not None and w.ant_name and \
                            w.ant_name.startswith(XFER):
                        tsi = trig.sync_info
                        tsi.on_wait = list(tsi.on_wait or []) + [w]
                        changed = True
                        n_xfer += 1
                    else:
                        new_waits.append(w)
                if changed:
                    si.on_wait = new_waits
                    n += 1
            block.instructions[:] = kept
    return n, n_del, n_xfer


# ------------------------------------------------------------ wait legalize
def legalize_waits(nc):
    """TRN2 TPB instructions have ONE sync-wait slot (EventSemaphore has 2);
    hoist extra waits left by the Tile scheduler into EVSEM prequels."""
    n_fixed = 0
    for func in nc.m.functions:
        for block in func.blocks:
            new_insts = []
            for inst in block.instructions:
                si = inst.sync_info
                waits = list(si.on_wait) if si and si.on_wait else []
                cap = 2 if isinstance(inst, mybir.InstEventSemaphore) else 1
                if isinstance(inst, mybir.InstDrain):
                    cap = 1
                if len(waits) > cap:
                    extra, keep = waits[:-cap], waits[-cap:]
                    for i in range(0, len(extra), 2):
                        new_insts.append(
                            mybir.InstEventSemaphore(
                                name=nc.get_next_instruction_name(),
                                ins=[],
                                outs=[],
                                engine=inst.engine,
                                sync_info=mybir.SyncInfo(
                                    on_wait=extra[i:i + 2], on_update=[]
                                ),
                            )
                        )
                    si.on_wait = keep
                    n_fixed += 1
                new_insts.append(inst)
            block.instructions[:] = new_insts
    return n_fixed


# ----------------------------------------------------------- host preprocess
def preprocess(edge_index):
    """Sort edges by dst, partition per core / per 128-dst block, split each
    block's edges into A/B-region runs (by source row within its owner core),
    pad counts to the per-block max across cores (program is SPMD-uniform).

    Each region's padded edge stream is then PACKED into gather calls of
    exactly CAP indices spanning block boundaries (the Q7 per-call fixed
    cost ~4us dominates, so call count is what matters).  Calls write 8
    consecutive subtiles of a 32-subtile ring arena per region; a block's
    indicator matmul consumes the (possibly boundary-shared) subtiles it
    touches, with foreign slots killed by dl=-1."""
    nb = (NC_N + BLK - 1) // BLK
    src = np.asarray(edge_index[0], np.int64)
    dst = np.asarray(edge_index[1], np.int64)
    order = np.argsort(dst, kind="stable")
    ds, ss = dst[order], src[order]

    core = ds // NC_N
    blk = (ds - core * NC_N) // BLK
    gblk = core * nb + blk
    n_gblk = N_CORES * nb
    bbounds = np.searchsorted(gblk, np.arange(n_gblk + 1))

    # source slot within the AllGather'd table regions
    sc = ss // NC_N
    r = ss - sc * NC_N
    ri = np.searchsorted(np.asarray(R_BOUNDS), r, side="right") - 1
    base_arr = np.asarray([R_BOUNDS[i] for i in range(NREG)])
    size_arr = np.asarray(R_SIZE)
    slot = sc * size_arr[ri] + (r - base_arr[ri])

    runs = {}    # (core, block, region) -> (slots, dls)
    n_r = np.zeros((NREG, N_CORES, nb), np.int64)
    for g in range(n_gblk):
        e0, e1 = bbounds[g], bbounds[g + 1]
        c, b = g // nb, g % nb
        base = c * NC_N + b * BLK
        sl, dl, rr = slot[e0:e1], ds[e0:e1] - base, ri[e0:e1]
        for i in range(NREG):
            m = rr == i
            s_i, d_i = sl[m], dl[m]
            # ascending slot order -> ascending HBM addresses
            o = np.argsort(s_i, kind="stable")
            runs[(c, b, i)] = (s_i[o], d_i[o])
            n_r[i, c, b] = len(s_i)

    n_u = n_r.max(axis=1).astype(int)     # [NREG, nb] uniform counts

    # region stream layout: block b's run occupies [start[b], start[b]+n)
    starts = []
    pads = []
    ncalls = []
    for i in range(NREG):
        st = np.concatenate([[0], np.cumsum(n_u[i])])
        total_pad = (int(st[-1]) + CAP - 1) // CAP * CAP
        starts.append(st)
        pads.append(total_pad)
        ncalls.append(total_pad // CAP)

    # per block: touched subtiles per region + dl columns
    blocks = []
    tot_s = 0
    for b in range(nb):
        entry = dict(dl_off=tot_s, subs=[], need=[0] * NREG)
        for i in range(NREG):
            e0, e1 = int(starts[i][b]), int(starts[i][b] + n_u[i][b])
            for s in range(e0 // BLK, (e1 + BLK - 1) // BLK):
                entry["subs"].append((i, s, e0, e1))
            entry["need"][i] = (e1 + CAP - 1) // CAP if e1 > 0 else 0
        entry["n_sub"] = len(entry["subs"])
        tot_s += entry["n_sub"]
        blocks.append(entry)

    # index planes: region streams wrapped per call (CAP idx = CAP//16 cols)
    col_off = [0]
    for i in range(NREG):
        col_off.append(col_off[-1] + ncalls[i] * (CAP // 16))
    w_idx = col_off[-1]
    idx_dev = np.zeros((N_CORES, 16, w_idx), np.int16)
    dl_dev = np.full((N_CORES, BLK, tot_s), -1.0, np.float32)

    for c in range(N_CORES):
        for i in range(NREG):
            streamv = np.zeros((pads[i],), np.int64)
            for b in range(nb):
                v, _ = runs[(c, b, i)]
                e0 = int(starts[i][b])
                streamv[e0:e0 + len(v)] = v
            # wrap16 whole region stream: idx k -> (p=k%16, col=k//16)
            idx_dev[c, :, col_off[i]:col_off[i] + pads[i] // 16] = \
                streamv.reshape(-1, 16).T.astype(np.int16)
        for b in range(nb):
            bl = blocks[b]
            for k, (i, s, e0, e1) in enumerate(bl["subs"]):
                _, dvals = runs[(c, b, i)]
                col = np.full((BLK,), -1.0, np.float32)
                lo = max(e0, s * BLK)
                hi = min(e0 + len(dvals), (s + 1) * BLK)
                if hi > lo:
                    col[lo - s * BLK:hi - s * BLK] = dvals[lo - e0:hi - e0]
                dl_dev[c, :, bl["dl_off"] + k] = col

    idx_full = np.tile(idx_dev, (1, 8, 1))     # replicate to 128 partitions
    s_max = max(bl["n_sub"] for bl in blocks)
    meta = dict(nb=nb, blocks=blocks, w_idx=w_idx, tot_s=tot_s, s_max=s_max,
                ncalls=ncalls, col_off=col_off)
    return idx_full, dl_dev.astype(BF), meta


def pack_weights(inp, s_max):
    """Host-side packing of the small replicated weight tensors (bf16)."""
    def bd(av):  # [H, 2F] -> block-diag [H*F, H] halves (query, msg)
        av = np.asarray(av, np.float32)
        q = np.zeros((H * F, H), np.float32)
        m = np.zeros((H * F, H), np.float32)
        for h in range(H):
            q[h * F:(h + 1) * F, h] = av[h, :F]
            m[h * F:(h + 1) * F, h] = av[h, F:]
        return q, m

    w = {}
    for l in (0, 1):
        w[f"Wr{l}"] = np.asarray(inp[f"Wr{l}"], np.float32).astype(BF)
        w[f"Wn{l}"] = np.asarray(inp[f"Wn{l}"], np.float32).astype(BF)
        w[f"Wa{l}"] = np.asarray(inp[f"Wa{l}"], np.float32).astype(BF)
        q_, m_ = bd(inp[f"av{l}"])
        w[f"avq{l}"], w[f"avm{l}"] = q_.astype(BF), m_.astype(BF)
        w[f"bn{l}"] = np.stack(
            [np.asarray(inp[f"g{l}"], np.float32),
             np.asarray(inp[f"b{l}"], np.float32)], axis=1)  # [64,2] f32
    w["headW"] = np.asarray(inp["head_W"], np.float32).astype(BF)
    w["headb"] = np.asarray(inp["head_b"], np.float32).reshape(3, 1)
    w["iota"] = np.broadcast_to(np.arange(BLK, dtype=np.float32),
                                (BLK, BLK)).astype(BF)
    w["iotaw"] = np.broadcast_to(
        np.tile(np.arange(BLK, dtype=np.float32), s_max),
        (BLK, s_max * BLK)).astype(BF)
    w["identb"] = np.eye(BLK, dtype=np.float32).astype(BF)
    w["identf"] = np.eye(BLK, dtype=np.float32)
    bo = np.zeros((H, H * F), np.float32)
    for h in range(H):
        bo[h, h * F:(h + 1) * F] = 1.0
    w["blkones"] = bo.astype(BF)
    return w


# ------------------------------------------------------------ device program
def build_program(meta):
    nb = meta["nb"]
    blocks = meta["blocks"]
    w_idx = meta["w_idx"]
    tot_s = meta["tot_s"]
    s_max = meta["s_max"]
    dims = [IN, F]

    nc = bacc.Bacc(None, num_swdge_queues=4)
    nc._gnn_prep_targets = {}   # prep inst name -> completion sem target
    nc._gnn_prep_trig = {}      # prep inst name -> its trigger (mybir inst)

    # ---- I/O
    xT = nc.declare_dram_parameter("xT", [IN, NC_N], BF16, isOutput=False)
    idx_in = nc.declare_dram_parameter("idx", [BLK, w_idx], I16, isOutput=False)
    dl_in = nc.declare_dram_parameter("dstloc", [BLK, tot_s], BF16, isOutput=False)
    wext = {}
    for l in (0, 1):
        d = dims[l]
        wext[f"Wr{l}"] = nc.declare_dram_parameter(f"Wr{l}", [d, F], BF16, isOutput=False)
        wext[f"Wn{l}"] = nc.declare_dram_parameter(f"Wn{l}", [d, H * F], BF16, isOutput=False)
        wext[f"Wa{l}"] = nc.declare_dram_parameter(f"Wa{l}", [d, H * F], BF16, isOutput=False)
        wext[f"avq{l}"] = nc.declare_dram_parameter(f"avq{l}", [H * F, H], BF16, isOutput=False)
        wext[f"avm{l}"] = nc.declare_dram_parameter(f"avm{l}", [H * F, H], BF16, isOutput=False)
        wext[f"bn{l}"] = nc.declare_dram_parameter(f"bn{l}", [F, 2], F32, isOutput=False)
    wext["headW"] = nc.declare_dram_parameter("headW", [F, 3], BF16, isOutput=False)
    wext["headb"] = nc.declare_dram_parameter("headb", [3, 1], F32, isOutput=False)
    wext["iota"] = nc.declare_dram_parameter("iota", [BLK, BLK], BF16, isOutput=False)
    wext["iotaw"] = nc.declare_dram_parameter("iotaw", [BLK, s_max * BLK], BF16, isOutput=False)
    wext["identb"] = nc.declare_dram_parameter("identb", [BLK, BLK], BF16, isOutput=False)
    wext["identf"] = nc.declare_dram_parameter("identf", [BLK, BLK], F32, isOutput=False)
    wext["blkones"] = nc.declare_dram_parameter("blkones", [H, H * F], BF16, isOutput=False)
    out_ext = nc.declare_dram_parameter("out", [3, NC_N], F32, isOutput=True)

    # ---- internal DRAM
    g_src = [nc.dram_tensor(f"g_src{l}", [NC_N, ROW], BF16) for l in (0, 1)]
    g_fullR = [[nc.dram_tensor(f"g_fullR{i}_{l}", [N_CORES * R_SIZE[i], ROW],
                               BF16, addr_space="Shared")
                for i in range(NREG)] for l in (0, 1)]
    bn_src = [nc.dram_tensor(f"bn_src{l}", [F, 2], F32) for l in (0, 1)]
    bn_out = [nc.dram_tensor(f"bn_out{l}", [F, 2], F32, addr_space="Shared")
              for l in (0, 1)]
    groups = [list(range(N_CORES))]

    n_chunks = (NC_N + CHUNK - 1) // CHUNK
    stage_cap = int(os.environ.get("GNN_STAGE", "9"))
    layer_cap = int(os.environ.get("GNN_LAYERS", "2"))

    # 8 rotating completion sems per queue: call k (per-queue ordinal) uses
    # slot k%8.  With <=8 calls in flight per queue (ring gates), each slot
    # has at most one call in flight, so the threshold 16*(k//8+1) is exact.
    dma_sems = [[nc.alloc_semaphore(f"gdma{q}_{j}") for j in range(8)]
                for q in range(4)]

    with tile.TileContext(nc) as tc:
        with contextlib.ExitStack() as ctx:
            cpool = ctx.enter_context(tc.tile_pool(name="const", bufs=1))
            wp = ctx.enter_context(tc.tile_pool(name="work", bufs=2))
            hp = ctx.enter_context(tc.tile_pool(name="resid", bufs=1))
            pp = ctx.enter_context(tc.tile_pool(name="psA", bufs=1, space="PSUM"))
            pb = ctx.enter_context(tc.tile_pool(name="psB", bufs=2, space="PSUM"))

            # ---- load constants
            wsb = {}
            for k, ext in wext.items():
                t = cpool.tile(list(ext.shape), ext.dtype, tag=k)
                nc.sync.dma_start(out=t[:], in_=ext[:])
                wsb[k] = t
            idx_sb = cpool.tile([BLK, w_idx], I16, tag="idx")
            nc.sync.dma_start(out=idx_sb[:], in_=idx_in[:])
            dl_sb = cpool.tile([BLK, tot_s], BF16, tag="dl")
            nc.sync.dma_start(out=dl_sb[:], in_=dl_in[:])

            hT_res = hp.tile([F, NC_N], F32, tag="hres")
            hT_act = hp.tile([F, NC_N], BF16, tag="hact")
            nc.vector.memset(hT_act[:], 0.0)
            arenas_sb = [hp.tile([BLK, ARENA_S, ROW], BF16, tag=f"ar{i}",
                                 name=f"arena{i}")
                         for i in range(NREG)]
            scr = hp.tile([F, BSPLIT], F32, tag="scr")
            stats = hp.tile([F, 8], F32, tag="stats")
            bnsc = hp.tile([F, 8], F32, tag="bnsc")

            for l in (0, 1)[:layer_cap]:
                d = dims[l]
                # ================= phase A: per-node G rows + x_root =======
                for ci in range(n_chunks):
                    c0 = ci * CHUNK
                    cw = min(CHUNK, NC_N - c0)
                    if l == 0:
                        rhs = wp.tile([IN, CHUNK], BF16, tag="xchunk")
                        nc.sync.dma_start(out=rhs[:, :cw], in_=xT[:, c0:c0 + cw])
                        rhs_ap = rhs[:IN, :cw]
                    else:
                        rhs_ap = hT_act[:F, c0:c0 + cw]

                    ps_jm = pp.tile([H * F, CHUNK], F32, tag="jm", space="PSUM")
                    ps_iq = pp.tile([H * F, CHUNK], F32, tag="iq", space="PSUM")
                    ps_r = pp.tile([F, CHUNK], F32, tag="r", space="PSUM")
                    nc.tensor.matmul(out=ps_jm[:, :cw], lhsT=wsb[f"Wn{l}"][:d, :],
                                     rhs=rhs_ap, start=True, stop=True)
                    nc.tensor.matmul(out=ps_iq[:, :cw], lhsT=wsb[f"Wa{l}"][:d, :],
                                     rhs=rhs_ap, start=True, stop=True)
                    nc.tensor.matmul(out=ps_r[:, :cw], lhsT=wsb[f"Wr{l}"][:d, :],
                                     rhs=rhs_ap, start=True, stop=True)
                    nc.vector.tensor_copy(hT_res[:, c0:c0 + cw], ps_r[:, :cw])

                    jm = wp.tile([H * F, CHUNK], BF16, tag="jm_sb")
                    nc.scalar.activation(jm[:, :cw], ps_jm[:, :cw], AF.Identity)
                    # leaky(x) = max(x, 0.2x)
                    lkjm = wp.tile([H * F, CHUNK], BF16, tag="lkjm")
                    nc.scalar.mul(lkjm[:, :cw], ps_jm[:, :cw], LEAKY)
                    nc.vector.tensor_tensor(out=lkjm[:, :cw], in0=lkjm[:, :cw],
                                            in1=jm[:, :cw], op=OP.max)
                    iq = wp.tile([H * F, CHUNK], BF16, tag="iq_sb")
                    nc.scalar.activation(iq[:, :cw], ps_iq[:, :cw], AF.Identity)
                    lkiq = wp.tile([H * F, CHUNK], BF16, tag="lkiq")
                    nc.scalar.mul(lkiq[:, :cw], ps_iq[:, :cw], LEAKY)
                    nc.vector.tensor_tensor(out=lkiq[:, :cw], in0=lkiq[:, :cw],
                                            in1=iq[:, :cw], op=OP.max)
                    ps_s = pp.tile([H, CHUNK], F32, tag="s", space="PSUM")
                    nc.tensor.matmul(out=ps_s[:, :cw], lhsT=wsb[f"avq{l}"][:],
                                     rhs=lkiq[:, :cw], start=True, stop=False)
                    nc.tensor.matmul(out=ps_s[:, :cw], lhsT=wsb[f"avm{l}"][:],
                                     rhs=lkjm[:, :cw], start=False, stop=True)
                    e_sb = wp.tile([H, CHUNK], BF16, tag="esb")
                    nc.scalar.activation(e_sb[:, :cw], ps_s[:, :cw], AF.Exp)
                    # broadcast E over the per-head 64 features via matmul
                    ps_eb = pp.tile([H * F, CHUNK], F32, tag="iq", space="PSUM")
                    nc.tensor.matmul(out=ps_eb[:, :cw], lhsT=wsb["blkones"][:],
                                     rhs=e_sb[:, :cw], start=True, stop=True)
                    eb = wp.tile([H * F, CHUNK], BF16, tag="eb")
                    nc.scalar.activation(eb[:, :cw], ps_eb[:, :cw], AF.Identity)
                    y = wp.tile([H * F, CHUNK], BF16, tag="y")
                    nc.vector.tensor_tensor(out=y[:, :cw], in0=jm[:, :cw],
                                            in1=eb[:, :cw], op=OP.mult)
                    # write G rows (transpose to node-major)
                    for q in range(0, cw, BLK):
                        qw = min(BLK, cw - q)
                        ps_t = pb.tile([BLK, BLK], BF16, tag="tp", space="PSUM")
                        nc.tensor.transpose(out=ps_t[:qw, :], in_=y[:, q:q + qw],
                                            identity=wsb["identb"][:])
                        ps_e = pb.tile([BLK, BLK], BF16, tag="tp", space="PSUM")
                        nc.tensor.transpose(out=ps_e[:qw, :H], in_=e_sb[:, q:q + qw],
                                            identity=wsb["identb"][:H, :H])
                        gt = wp.tile([BLK, ROW], BF16, tag="gt")
                        nc.vector.tensor_copy(gt[:qw, 0:H * F], ps_t[:qw, :])
                        nc.vector.tensor_copy(gt[:qw, H * F:GVAL], ps_e[:qw, :H])
                        nc.sync.dma_start(
                            out=g_src[l][c0 + q:c0 + q + qw, :],
                            in_=gt[:qw, :])
                    # AllGather each region as soon as its rows are written
                    if stage_cap >= 2:
                        for i in range(NREG):
                            if ci == AG_CHUNK[i]:
                                nc.gpsimd.collective_compute(
                                    "AllGather", OP.bypass,
                                    replica_groups=groups,
                                    ins=[g_src[l][R_BOUNDS[i]:R_BOUNDS[i + 1], :]],
                                    outs=[g_fullR[l][i][:]])

                if stage_cap < 2:
                    continue

                # ================= phase B: gather + indicator matmul ======
                if stage_cap < 3:
                    continue
                # Synchronous gather calls are the DEFAULT: the Q7's desc-gen
                # is cheaper in immediate mode (~6.9 vs 8.3 ns/idx) and the
                # DMA overlaps later calls via the 4-queue rotation anyway.
                sync_mode = not os.environ.get("GNN_ASYNC")
                if l == 0:
                    cum_calls = [0, 0, 0, 0]   # per-queue call ordinals
                    prev_prep = [None, None, None, None]
                    prev_trigger = [None, None, None, None]
                    trig_of = {}               # (q, ordinal) -> trigger inst
                    call_ctr = [0]
                emitted = [0] * NREG
                call_trig = {}

                def chain(inst, *prevs):
                    deps = bass.InstructionNameOrderedSet()
                    have = False
                    for pv in prevs:
                        if pv is not None:
                            deps.add(pv.ins.name)
                            have = True
                    if have:
                        inst.ins.add_nosync_dependencies_from(deps)

                def emit_call(reg, k):
                    q = call_ctr[0] % 4
                    call_ctr[0] += 1
                    col0 = meta["col_off"][reg] + k * (CAP // 16)
                    in_view = g_fullR[l][reg][:]
                    arena = arenas_sb[reg]
                    slot0 = (8 * k) % ARENA_S
                    if sync_mode:
                        nc.gpsimd.dma_gather(
                            out_ap=arena[:, slot0:slot0 + 8, :],
                            in_ap=in_view,
                            idxs_ap=idx_sb[:, col0:col0 + CAP // 16],
                            num_idxs=CAP, num_idxs_reg=CAP,
                            elem_size=ROW, queue_num=q)
                        call_trig[(reg, k)] = None
                        return
                    ordinal = cum_calls[q]
                    slot = ordinal % 8
                    gate = None
                    if ordinal >= GATE_D:
                        # ring-capacity gate: call (ordinal-GATE_D) of this
                        # queue must be fully drained -> at most GATE_D calls
                        # (~260 descs/engine of the ring) in flight per
                        # queue, and the 8 sem slots stay unambiguous
                        og = ordinal - GATE_D
                        gate = nc.gpsimd.wait_ge(dma_sems[q][og % 8],
                                                 16 * (og // 8 + 1))
                        chain(gate, trig_of[(q, og)], prev_prep[q])
                    p = nc.gpsimd.dma_gather(
                        out_ap=arena[:, slot0:slot0 + 8, :],
                        in_ap=in_view,
                        idxs_ap=idx_sb[:, col0:col0 + CAP // 16],
                        num_idxs=CAP, num_idxs_reg=CAP,
                        elem_size=ROW, queue_num=q,
                        prepare_only=True, sem=dma_sems[q][slot])
                    nc._gnn_prep_targets[p.ins.name] = 16 * (ordinal // 8 + 1)
                    chain(p, gate, prev_prep[q])
                    prev_prep[q] = p
                    t = nc.gpsimd.trigger_dma(count=1, queue_num=q)
                    chain(t, p, prev_trigger[q])
                    nc._gnn_prep_trig[p.ins.name] = t.ins
                    prev_trigger[q] = t
                    trig_of[(q, ordinal)] = t
                    call_trig[(reg, k)] = t
                    cum_calls[q] = ordinal + 1

                for b in range(nb):
                    bl = blocks[b]
                    b0 = b * BLK
                    bw = min(BLK, NC_N - b0)
                    for i in range(NREG):
                        while emitted[i] < bl["need"][i]:
                            emit_call(i, emitted[i])
                            emitted[i] += 1
                    if stage_cap < 4:
                        continue
                    off = bl["dl_off"]
                    n_sub = bl["n_sub"]
                    ind = wp.tile([BLK, s_max * BLK], BF16, tag="ind", bufs=3)
                    nc.vector.tensor_tensor(
                        out=ind[:, 0:n_sub * BLK].rearrange("p (s i) -> p s i", i=BLK),
                        in0=dl_sb[:, off:off + n_sub][:, :, None]
                            .to_broadcast([BLK, n_sub, BLK]),
                        in1=wsb["iotaw"][:, 0:n_sub * BLK]
                            .rearrange("p (s i) -> p s i", i=BLK),
                        op=OP.is_equal)
                    ps_blk = pb.tile([BLK, GVAL], F32, tag="blk", space="PSUM")
                    for j, (reg, s, e0, e1) in enumerate(bl["subs"]):
                        arena = arenas_sb[reg]
                        mm = nc.tensor.matmul(out=ps_blk[:],
                                              lhsT=ind[:, j * BLK:(j + 1) * BLK],
                                              rhs=arena[:, s % ARENA_S, 0:GVAL],
                                              start=(j == 0), stop=(j == n_sub - 1))
                        tg = call_trig.get((reg, s // 8))
                        if mm is not None and tg is not None:
                            # scheduling-order (no-sync) edge: keep stage
                            # consumers after their call's trigger in the PE
                            # stream, else PE head-of-line blocks on data
                            # whose trigger hasn't dispatched yet
                            deps = bass.InstructionNameOrderedSet()
                            deps.add(tg.ins.name)
                            mm.ins.add_nosync_dependencies_from(deps)
                    sb = wp.tile([BLK, GVAL], F32, tag="sbblk")
                    nc.vector.tensor_copy(sb[:], ps_blk[:])
                    rec = wp.tile([BLK, H], F32, tag="rec")
                    nc.vector.tensor_scalar_add(rec[:], sb[:, H * F:GVAL], 1e-30)
                    nc.vector.reciprocal(rec[:], rec[:])
                    agg = wp.tile([BLK, F], F32, tag="agg")
                    tmp = wp.tile([BLK, F], F32, tag="tmp")
                    nc.scalar.activation(agg[:], sb[:, 0:F], AF.Identity,
                                         scale=rec[:, 0:1])
                    nc.scalar.activation(tmp[:], sb[:, F:2 * F], AF.Identity,
                                         scale=rec[:, 1:2])
                    nc.vector.tensor_add(out=agg[:], in0=agg[:], in1=tmp[:])
                    agg_bf = wp.tile([BLK, F], BF16, tag="aggbf")
                    nc.vector.tensor_copy(agg_bf[:], agg[:])
                    ps_t = pb.tile([BLK, BLK], BF16, tag="tp", space="PSUM")
                    nc.tensor.transpose(out=ps_t[:F, :], in_=agg_bf[:, :F],
                                        identity=wsb["identb"][:])
                    nc.vector.tensor_add(out=hT_res[:, b0:b0 + bw],
                                         in0=hT_res[:, b0:b0 + bw],
                                         in1=ps_t[:F, :bw])

                # ================= BatchNorm + ReLU ========================
                if stage_cap < 5:
                    continue
                # stats in two block-aligned halves so the first can reduce
                # while phase B still works on the second half's blocks
                half = ASPLIT
                nc.vector.reduce_sum(out=stats[:, 0:1], in_=hT_res[:, 0:half],
                                     axis=mybir.AxisListType.X)
                nc.scalar.square(scr[:, 0:half], hT_res[:, 0:half])
                nc.vector.reduce_sum(out=stats[:, 1:2], in_=scr[:, 0:half],
                                     axis=mybir.AxisListType.X)
                nc.vector.reduce_sum(out=stats[:, 4:5],
                                     in_=hT_res[:, half:NC_N],
                                     axis=mybir.AxisListType.X)
                nc.scalar.square(scr[:, 0:NC_N - half], hT_res[:, half:NC_N])
                nc.vector.reduce_sum(out=stats[:, 5:6], in_=scr[:, 0:NC_N - half],
                                     axis=mybir.AxisListType.X)
                nc.vector.tensor_add(out=stats[:, 0:1], in0=stats[:, 0:1],
                                     in1=stats[:, 4:5])
                nc.vector.tensor_add(out=stats[:, 1:2], in0=stats[:, 1:2],
                                     in1=stats[:, 5:6])
                nc.sync.dma_start(out=bn_src[l][:], in_=stats[:, 0:2])
                nc.gpsimd.collective_compute(
                    "AllReduce", OP.add, replica_groups=groups,
                    ins=[bn_src[l][:]], outs=[bn_out[l][:]])
                nc.sync.dma_start(out=stats[:, 2:4], in_=bn_out[l][:])
                nc.scalar.mul(bnsc[:, 0:1], stats[:, 2:3], 1.0 / N)
                nc.scalar.mul(bnsc[:, 1:2], stats[:, 3:4], 1.0 / N)
                nc.vector.tensor_tensor(out=bnsc[:, 2:3], in0=bnsc[:, 0:1],
                                        in1=bnsc[:, 0:1], op=OP.mult)
                nc.vector.tensor_tensor(out=bnsc[:, 2:3], in0=bnsc[:, 1:2],
                                        in1=bnsc[:, 2:3], op=OP.subtract)
                nc.vector.tensor_scalar_add(bnsc[:, 2:3], bnsc[:, 2:3], BN_EPS)
                nc.vector.reciprocal(bnsc[:, 3:4], bnsc[:, 2:3])
                nc.scalar.sqrt(bnsc[:, 4:5], bnsc[:, 3:4])
                nc.vector.tensor_tensor(out=bnsc[:, 5:6], in0=bnsc[:, 4:5],
                                        in1=wsb[f"bn{l}"][:, 0:1], op=OP.mult)
                nc.vector.tensor_tensor(out=bnsc[:, 6:7], in0=bnsc[:, 0:1],
                                        in1=bnsc[:, 5:6], op=OP.mult)
                nc.vector.tensor_tensor(out=bnsc[:, 6:7], in0=wsb[f"bn{l}"][:, 1:2],
                                        in1=bnsc[:, 6:7], op=OP.subtract)
                # apply per chunk so the next layer / head can start on
                # early chunks while later ones are still being written
                for ci in range(n_chunks):
                    c0 = ci * CHUNK
                    cw = min(CHUNK, NC_N - c0)
                    nc.scalar.activation(hT_act[:, c0:c0 + cw],
                                         hT_res[:, c0:c0 + cw],
                                         AF.Relu, bias=bnsc[:, 6:7],
                                         scale=bnsc[:, 5:6])

            # ================= head ========================================
            for ci in range(n_chunks):
                c0 = ci * CHUNK
                cw = min(CHUNK, NC_N - c0)
                ps_o = pp.tile([3, CHUNK], F32, tag="s", space="PSUM")
                nc.tensor.matmul(out=ps_o[:, :cw], lhsT=wsb["headW"][:],
                                 rhs=hT_act[:F, c0:c0 + cw], start=True, stop=True)
                osb = wp.tile([3, CHUNK], F32, tag="osb")
                nc.scalar.activation(osb[:, :cw], ps_o[:, :cw],
                                     AF.Identity, bias=wsb["headb"][:, 0:1])
                nc.sync.dma_start(out=out_ext[:, c0:c0 + cw], in_=osb[:, :cw])

    return nc


# ---------------------------------------------------------------- run cache
_CACHE = {}


def _build_inputs(inputs, meta, idx_full, dl_dev):
    w = pack_weights(inputs, meta["s_max"])
    x = np.asarray(inputs["x"], np.float32)
    in_maps = []
    for c in range(N_CORES):
        m = dict(w)
        m["xT"] = np.ascontiguousarray(
            x[c * NC_N:(c + 1) * NC_N, :].T).astype(BF)
        m["idx"] = np.ascontiguousarray(idx_full[c])
        m["dstloc"] = np.ascontiguousarray(dl_dev[c])
        in_maps.append(m)
    return in_maps


def kernel(**inputs):
    from concourse.bass_utils import run_bass_kernel_spmd

    _install_hookshim()
    edge = np.asarray(inputs["edge_index"])
    key = hashlib.sha1(edge.tobytes()).hexdigest()
    if key not in _CACHE:
        idx_full, dl_dev, meta = preprocess(edge)
        nc = build_program(meta)
        nc.finalize()
        if os.environ.get("GNN_ASYNC"):
            n_remap, n_del, n_xfer = remap_dmasw_waits(nc)
            print(f"remapped DMASW waits on {n_remap} insts, deleted "
                  f"{n_del} IncSwdgeSem, moved {n_xfer} waits to triggers")
        n_fix = legalize_waits(nc)
        if n_fix:
            print(f"legalize_waits fixed {n_fix} instructions post-finalize")
        _CACHE[key] = (idx_full, dl_dev, meta, nc)
    idx_full, dl_dev, meta, nc = _CACHE[key]
    in_maps = _build_inputs(inputs, meta, idx_full, dl_dev)
    res = run_bass_kernel_spmd(
        nc, in_maps, list(range(N_CORES)),
        trace=bool(os.environ.get("GNN_TRACE")))
    if res.exec_time_ns is not None:
        print(f"HW exec time: {res.exec_time_ns} ns")
    out = np.concatenate([res.results[c]["out"] for c in range(N_CORES)],
                         axis=1)  # [3, N]
    return np.ascontiguousarray(out.T).astype(np.float32)


# revision 75
# speedup vs baseline: 1.2126x; 1.2126x over previous
"""AttnGraphSAGE on 8 Trainium2 NeuronCores (Bass/Tile) — v2.

Math restructuring (unchanged from v1): attention logits depend only on the
SOURCE node, so the whole edge phase is ONE segment-sum over dst of per-src
rows G[n] = [E_0*x_jm_0 (64) | E_1*x_jm_1 (64) | E_0 | E_1] (130 values).

v2 performance changes (2268us -> ~1274us on 8 cores):
  * G rows are bf16, 256-elem / 512B strides (was f32 768B): halves the
    random-gather HBM traffic and the AllGather volume.  All matmul operands
    (weights, activations, indicator) are bf16 -> 1-pass PE instead of 4.
  * The G table is AllGather'd in FOUR per-core row regions; each region's
    collective is issued as soon as phase A finishes its rows, so early
    regions' gathers overlap both the rest of phase A and the later
    AllGathers.  Every region stays < 32768 total rows, so region base
    addresses double as the int16-index split.
  * Gather calls are PACKED to exactly 1024 indices (the Q7 per-call
    ceiling) spanning dst-block boundaries; each call fills 8 consecutive
    subtiles of a 32-subtile ring arena per region.  A block's indicator
    matmuls consume the subtiles it touches; boundary subtiles are consumed
    by both adjacent blocks with foreign slots killed by dstloc=-1.  This
    minimizes Q7 descriptor-generation calls (~7ns/idx on the critical
    engine) with zero padding waste.
  * Synchronous (immediate) gather calls: measured cheaper per idx than the
    PREPARE_ONLY+trigger path, and the 4-queue rotation overlaps the DMA
    with subsequent descriptor generation anyway.  (GNN_ASYNC=1 selects the
    prepare/trigger path, kept for experiments.)
  * PSUM->bf16 casts and the per-head 1/denom scaling run on the Scalar
    engine (per-partition scale operand), and the is_equal indicator build
    compares against a materialized iota operand, unloading the DVE which
    is co-critical with the Q7/DMA during the edge phase.
  * BN stats reduce in block-aligned halves (first half starts during the
    phase-B tail) and the BN ReLU applies per chunk so the next layer's
    matmuls start on early chunks.
  * Per-core counts padded only to the max across the 8 cores so the
    program stays SPMD-uniform; 0-padded (no trailing -1 indices).
"""
import os
import sys
import types
import hashlib
import contextlib

sys.path.insert(0, "/opt/trn_rl_repo")

import numpy as np
import ml_dtypes

import concourse.bass as bass
import concourse.bacc as bacc
import concourse.mybir as mybir
from concourse import tile

# ---------------------------------------------------------------- constants
N = 50000
E = 800000
IN = 128
F = 64
H = 2
N_CORES = 8
NC_N = N // N_CORES          # 6250 nodes per core
BLK = 128                    # dst nodes per block
ROW = 256                    # G row stride in bf16 elems (512B)
GVAL = 2 * F + H             # 130 used cols
CHUNK = 512                  # phase-A node chunk
# G-table AllGather regions (per-core row ranges).  4 regions so the first
# regions' gathers start while later regions are still being computed /
# AllGather'd; each region stays < 32768 total rows for int16 indices.
R_BOUNDS = [0, 1536, 3072, 4608, NC_N]
NREG = len(R_BOUNDS) - 1
R_SIZE = [R_BOUNDS[i + 1] - R_BOUNDS[i] for i in range(NREG)]
AG_CHUNK = [(R_BOUNDS[i + 1] + CHUNK - 1) // CHUNK - 1 for i in range(NREG)]
ASPLIT = 3072                # BN stats half split (block-aligned)
BSPLIT = NC_N - ASPLIT
CAP = int(os.environ.get("GNN_CAP", "1024"))   # idxs per gather call (HW max)
ARENA_S = 32                 # ring-arena subtiles per region (4 calls)
GATE_D = 4                   # calls in flight per queue (ring + sem-slot cap)
F32 = mybir.dt.float32
BF16 = mybir.dt.bfloat16
FP8 = mybir.dt.float8e4
I16 = mybir.dt.int16
AF = mybir.ActivationFunctionType
OP = mybir.AluOpType
BN_EPS = 1e-5
LEAKY = 0.2
BF = ml_dtypes.bfloat16


# ------------------------------------------------------- axon profile shim
def _install_hookshim():
    if "antenv.axon_hooks" in sys.modules:
        return
    mod = types.ModuleType("antenv.axon_hooks")
    _h = [None]
    mod.set_axon_ntff_profile_hook = lambda h: _h.__setitem__(0, h)
    mod.get_axon_ntff_profile_hook = lambda: _h[0]
    try:
        import antenv
        sys.modules["antenv.axon_hooks"] = mod
        antenv.axon_hooks = mod
        from trn_agent_boot.trn_boot import _ntff_profile_via_ctypes
        mod.set_axon_ntff_profile_hook(
            _ntff_profile_via_ctypes("/opt/axon/libaxon_pjrt.so")
        )
    except Exception:
        pass


def remap_dmasw_waits(nc):
    """Remap waits on Tile's DMASW lane semaphores to the per-queue gather
    DMA-completion sems.

    Tile assigned each PREPARE_ONLY gather prep a DMASW lane (round-robin)
    and derived all downstream waits (consumers, ring flow control) as
    ``DMASW{lane} >= 16*tick``.  But the sem actually baked into the
    descriptors (and bumped by the SDMA engines) is our per-queue gdma sem,
    so those lane sems never move.  Each prep records its assigned
    (lane proc, tick); since each queue's ring is FIFO, the k-th prep of
    queue q has completed exactly when gdma{q} >= 16*k.  Rewrite every
    DMASW wait for (lane, tick) into the equivalent (and race-free)
    per-queue wait."""
    from concourse.tile_sem_assignment import PROC_NAME_TO_IDX
    inv_proc = {v: k for k, v in PROC_NAME_TO_IDX.items()}

    insts = []
    for func in nc.m.functions:
        for block in func.blocks:
            insts.extend(block.instructions)

    # (lane_name, 16*tick) -> (gdma sem id, gdma name, block-level target)
    lane_map = {}
    for inst in insts:
        if type(inst).__name__ == "InstDMAGatherAnt" and \
                getattr(inst, "gen_mode", 0) == 1:
            lane = inv_proc[inst.bass_scheduled_proc]
            upd = inst.sync_info.on_update[0]
            assert upd.ant_name.startswith("gdma"), upd.ant_name
            key = (lane, 16 * inst.bass_scheduled_tick)
            assert key not in lane_map, key
            lane_map[key] = (upd.id, upd.ant_name,
                             nc._gnn_prep_targets[inst.name])

    # waits with these prefixes are deferred from a prep to its trigger:
    # the prep only writes ring descriptors; the DMA (which actually touches
    # the arena / g_full) fires at the trigger, so enforcing reader-WAR and
    # collective deps there frees desc-gen to run ahead.
    XFER = ("PE_", "DVE_", "Act", "Collectives_")
    n = 0
    n_del = 0
    n_xfer = 0
    for func in nc.m.functions:
        for block in func.blocks:
            kept = []
            for inst in block.instructions:
                # Tile's per-prep DMASW shadow-sem maintenance is dead weight
                # once nothing uses the lane sems (1.65us of Pool each, plus
                # serializing ring-drain waits); the ring-capacity gates keep
                # the ring below capacity without it.
                if type(inst).__name__ == "InstIncSwdgeSem":
                    n_del += 1
                    continue
                kept.append(inst)
                si = inst.sync_info
                if not (si and si.on_wait):
                    continue
                changed = False
                new_waits = []
                trig = nc._gnn_prep_trig.get(inst.name)
                for w in si.on_wait:
                    if w.ant_name and w.ant_name.startswith("DMASW"):
                        lane = w.ant_name.split("_")[0]
                        sid, sname, thresh = lane_map[(lane, w.wait_value)]
                        new_waits.append(mybir.SyncWait(
                            sync_type="semaphore", id=sid,
                            wait_mode="sem-ge-imm",
                            wait_value=thresh, ant_name=sname))
                        changed = True
                    elif trig is not None and w.ant_name and \
                            w.ant_name.startswith(XFER):
                        tsi = trig.sync_info
                        tsi.on_wait = list(tsi.on_wait or []) + [w]
                        changed = True
                        n_xfer += 1
                    else:
                        new_waits.append(w)
                if changed:
                    si.on_wait = new_waits
                    n += 1
            block.instructions[:] = kept
    return n, n_del, n_xfer


# ------------------------------------------------------------ wait legalize
def legalize_waits(nc):
    """TRN2 TPB instructions have ONE sync-wait slot (EventSemaphore has 2);
    hoist extra waits left by the Tile scheduler into EVSEM prequels."""
    n_fixed = 0
    for func in nc.m.functions:
        for block in func.blocks:
            new_insts = []
            for inst in block.instructions:
                si = inst.sync_info
                waits = list(si.on_wait) if si and si.on_wait else []
                cap = 2 if isinstance(inst, mybir.InstEventSemaphore) else 1
                if isinstance(inst, mybir.InstDrain):
                    cap = 1
                if len(waits) > cap:
                    extra, keep = waits[:-cap], waits[-cap:]
                    for i in range(0, len(extra), 2):
                        new_insts.append(
                            mybir.InstEventSemaphore(
                                name=nc.get_next_instruction_name(),
                                ins=[],
                                outs=[],
                                engine=inst.engine,
                                sync_info=mybir.SyncInfo(
                                    on_wait=extra[i:i + 2], on_update=[]
                                ),
                            )
                        )
                    si.on_wait = keep
                    n_fixed += 1
                new_insts.append(inst)
            block.instructions[:] = new_insts
    return n_fixed


# ----------------------------------------------------------- host preprocess
def preprocess(edge_index):
    """Sort edges by dst, partition per core / per 128-dst block, split each
    block's edges into A/B-region runs (by source row within its owner core),
    pad counts to the per-block max across cores (program is SPMD-uniform).

    Each region's padded edge stream is then PACKED into gather calls of
    exactly CAP indices spanning block boundaries (the Q7 per-call fixed
    cost ~4us dominates, so call count is what matters).  Calls write 8
    consecutive subtiles of a 32-subtile ring arena per region; a block's
    indicator matmul consumes the (possibly boundary-shared) subtiles it
    touches, with foreign slots killed by dl=-1."""
    nb = (NC_N + BLK - 1) // BLK
    src = np.asarray(edge_index[0], np.int64)
    dst = np.asarray(edge_index[1], np.int64)
    order = np.argsort(dst, kind="stable")
    ds, ss = dst[order], src[order]

    core = ds // NC_N
    blk = (ds - core * NC_N) // BLK
    gblk = core * nb + blk
    n_gblk = N_CORES * nb
    bbounds = np.searchsorted(gblk, np.arange(n_gblk + 1))

    # source slot within the AllGather'd table regions
    sc = ss // NC_N
    r = ss - sc * NC_N
    ri = np.searchsorted(np.asarray(R_BOUNDS), r, side="right") - 1
    base_arr = np.asarray([R_BOUNDS[i] for i in range(NREG)])
    size_arr = np.asarray(R_SIZE)
    slot = sc * size_arr[ri] + (r - base_arr[ri])

    runs = {}    # (core, block, region) -> (slots, dls)
    n_r = np.zeros((NREG, N_CORES, nb), np.int64)
    for g in range(n_gblk):
        e0, e1 = bbounds[g], bbounds[g + 1]
        c, b = g // nb, g % nb
        base = c * NC_N + b * BLK
        sl, dl, rr = slot[e0:e1], ds[e0:e1] - base, ri[e0:e1]
        for i in range(NREG):
            m = rr == i
            s_i, d_i = sl[m], dl[m]
            # ascending slot order -> ascending HBM addresses
            o = np.argsort(s_i, kind="stable")
            runs[(c, b, i)] = (s_i[o], d_i[o])
            n_r[i, c, b] = len(s_i)

    n_u = n_r.max(axis=1).astype(int)     # [NREG, nb] uniform counts

    # region stream layout: block b's run occupies [start[b], start[b]+n)
    starts = []
    pads = []
    ncalls = []
    for i in range(NREG):
        st = np.concatenate([[0], np.cumsum(n_u[i])])
        total_pad = (int(st[-1]) + CAP - 1) // CAP * CAP
        starts.append(st)
        pads.append(total_pad)
        ncalls.append(total_pad // CAP)

    # per block: touched subtiles per region + dl columns
    blocks = []
    tot_s = 0
    for b in range(nb):
        entry = dict(dl_off=tot_s, subs=[], need=[0] * NREG)
        for i in range(NREG):
            e0, e1 = int(starts[i][b]), int(starts[i][b] + n_u[i][b])
            for s in range(e0 // BLK, (e1 + BLK - 1) // BLK):
                entry["subs"].append((i, s, e0, e1))
            entry["need"][i] = (e1 + CAP - 1) // CAP if e1 > 0 else 0
        entry["n_sub"] = len(entry["subs"])
        tot_s += entry["n_sub"]
        blocks.append(entry)

    # index planes: region streams wrapped per call (CAP idx = CAP//16 cols)
    col_off = [0]
    for i in range(NREG):
        col_off.append(col_off[-1] + ncalls[i] * (CAP // 16))
    w_idx = col_off[-1]
    idx_dev = np.zeros((N_CORES, 16, w_idx), np.int16)
    dl_dev = np.full((N_CORES, BLK, tot_s), -1.0, np.float32)

    for c in range(N_CORES):
        for i in range(NREG):
            streamv = np.zeros((pads[i],), np.int64)
            for b in range(nb):
                v, _ = runs[(c, b, i)]
                e0 = int(starts[i][b])
                streamv[e0:e0 + len(v)] = v
            # wrap16 whole region stream: idx k -> (p=k%16, col=k//16)
            idx_dev[c, :, col_off[i]:col_off[i] + pads[i] // 16] = \
                streamv.reshape(-1, 16).T.astype(np.int16)
        for b in range(nb):
            bl = blocks[b]
            for k, (i, s, e0, e1) in enumerate(bl["subs"]):
                _, dvals = runs[(c, b, i)]
                col = np.full((BLK,), -1.0, np.float32)
                lo = max(e0, s * BLK)
                hi = min(e0 + len(dvals), (s + 1) * BLK)
                if hi > lo:
                    col[lo - s * BLK:hi - s * BLK] = dvals[lo - e0:hi - e0]
                dl_dev[c, :, bl["dl_off"] + k] = col

    idx_full = np.tile(idx_dev, (1, 8, 1))     # replicate to 128 partitions
    s_max = max(bl["n_sub"] for bl in blocks)
    meta = dict(nb=nb, blocks=blocks, w_idx=w_idx, tot_s=tot_s, s_max=s_max,
                ncalls=ncalls, col_off=col_off)
    return idx_full, dl_dev.astype(BF), meta


def pack_weights(inp, s_max):
    """Host-side packing of the small replicated weight tensors (bf16)."""
    def bd(av):  # [H, 2F] -> block-diag [H*F, H] halves (query, msg)
        av = np.asarray(av, np.float32)
        q = np.zeros((H * F, H), np.float32)
        m = np.zeros((H * F, H), np.float32)
        for h in range(H):
            q[h * F:(h + 1) * F, h] = av[h, :F]
            m[h * F:(h + 1) * F, h] = av[h, F:]
        return q, m

    w = {}
    for l in (0, 1):
        w[f"Wr{l}"] = np.asarray(inp[f"Wr{l}"], np.float32).astype(BF)
        w[f"Wn{l}"] = np.asarray(inp[f"Wn{l}"], np.float32).astype(BF)
        w[f"Wa{l}"] = np.asarray(inp[f"Wa{l}"], np.float32).astype(BF)
        q_, m_ = bd(inp[f"av{l}"])
        w[f"avq{l}"], w[f"avm{l}"] = q_.astype(BF), m_.astype(BF)
        w[f"bn{l}"] = np.stack(
            [np.asarray(inp[f"g{l}"], np.float32),
             np.asarray(inp[f"b{l}"], np.float32)], axis=1)  # [64,2] f32
    w["headW"] = np.asarray(inp["head_W"], np.float32).astype(BF)
    w["headb"] = np.asarray(inp["head_b"], np.float32).reshape(3, 1)
    w["iota"] = np.broadcast_to(np.arange(BLK, dtype=np.float32),
                                (BLK, BLK)).astype(BF)
    w["iotaw"] = np.broadcast_to(
        np.tile(np.arange(BLK, dtype=np.float32), s_max),
        (BLK, s_max * BLK)).astype(BF)
    w["identb"] = np.eye(BLK, dtype=np.float32).astype(BF)
    w["identf"] = np.eye(BLK, dtype=np.float32)
    bo = np.zeros((H, H * F), np.float32)
    for h in range(H):
        bo[h, h * F:(h + 1) * F] = 1.0
    w["blkones"] = bo.astype(BF)
    return w


# ------------------------------------------------------------ device program
def build_program(meta):
    nb = meta["nb"]
    blocks = meta["blocks"]
    w_idx = meta["w_idx"]
    tot_s = meta["tot_s"]
    s_max = meta["s_max"]
    dims = [IN, F]

    nc = bacc.Bacc(None, num_swdge_queues=4)
    nc._gnn_prep_targets = {}   # prep inst name -> completion sem target
    nc._gnn_prep_trig = {}      # prep inst name -> its trigger (mybir inst)

    # ---- I/O
    xT = nc.declare_dram_parameter("xT", [IN, NC_N], BF16, isOutput=False)
    idx_in = nc.declare_dram_parameter("idx", [BLK, w_idx], I16, isOutput=False)
    dl_in = nc.declare_dram_parameter("dstloc", [BLK, tot_s], BF16, isOutput=False)
    wext = {}
    for l in (0, 1):
        d = dims[l]
        wext[f"Wr{l}"] = nc.declare_dram_parameter(f"Wr{l}", [d, F], BF16, isOutput=False)
        wext[f"Wn{l}"] = nc.declare_dram_parameter(f"Wn{l}", [d, H * F], BF16, isOutput=False)
        wext[f"Wa{l}"] = nc.declare_dram_parameter(f"Wa{l}", [d, H * F], BF16, isOutput=False)
        wext[f"avq{l}"] = nc.declare_dram_parameter(f"avq{l}", [H * F, H], BF16, isOutput=False)
        wext[f"avm{l}"] = nc.declare_dram_parameter(f"avm{l}", [H * F, H], BF16, isOutput=False)
        wext[f"bn{l}"] = nc.declare_dram_parameter(f"bn{l}", [F, 2], F32, isOutput=False)
    wext["headW"] = nc.declare_dram_parameter("headW", [F, 3], BF16, isOutput=False)
    wext["headb"] = nc.declare_dram_parameter("headb", [3, 1], F32, isOutput=False)
    wext["iota"] = nc.declare_dram_parameter("iota", [BLK, BLK], BF16, isOutput=False)
    wext["iotaw"] = nc.declare_dram_parameter("iotaw", [BLK, s_max * BLK], BF16, isOutput=False)
    wext["identb"] = nc.declare_dram_parameter("identb", [BLK, BLK], BF16, isOutput=False)
    wext["identf"] = nc.declare_dram_parameter("identf", [BLK, BLK], F32, isOutput=False)
    wext["blkones"] = nc.declare_dram_parameter("blkones", [H, H * F], BF16, isOutput=False)
    out_ext = nc.declare_dram_parameter("out", [3, NC_N], F32, isOutput=True)

    # ---- internal DRAM
    g_src = [nc.dram_tensor(f"g_src{l}", [NC_N, ROW], BF16) for l in (0, 1)]
    g_fullR = [[nc.dram_tensor(f"g_fullR{i}_{l}", [N_CORES * R_SIZE[i], ROW],
                               BF16, addr_space="Shared")
                for i in range(NREG)] for l in (0, 1)]
    bn_src = [nc.dram_tensor(f"bn_src{l}", [F, 2], F32) for l in (0, 1)]
    bn_out = [nc.dram_tensor(f"bn_out{l}", [F, 2], F32, addr_space="Shared")
              for l in (0, 1)]
    groups = [list(range(N_CORES))]

    n_chunks = (NC_N + CHUNK - 1) // CHUNK
    stage_cap = int(os.environ.get("GNN_STAGE", "9"))
    layer_cap = int(os.environ.get("GNN_LAYERS", "2"))

    # 8 rotating completion sems per queue: call k (per-queue ordinal) uses
    # slot k%8.  With <=8 calls in flight per queue (ring gates), each slot
    # has at most one call in flight, so the threshold 16*(k//8+1) is exact.
    dma_sems = [[nc.alloc_semaphore(f"gdma{q}_{j}") for j in range(8)]
                for q in range(4)]

    with tile.TileContext(nc) as tc:
        with contextlib.ExitStack() as ctx:
            cpool = ctx.enter_context(tc.tile_pool(name="const", bufs=1))
            wp = ctx.enter_context(tc.tile_pool(name="work", bufs=2))
            hp = ctx.enter_context(tc.tile_pool(name="resid", bufs=1))
            pp = ctx.enter_context(tc.tile_pool(name="psA", bufs=1, space="PSUM"))
            pb = ctx.enter_context(tc.tile_pool(name="psB", bufs=2, space="PSUM"))

            # ---- load constants
            wsb = {}
            for k, ext in wext.items():
                t = cpool.tile(list(ext.shape), ext.dtype, tag=k)
                nc.sync.dma_start(out=t[:], in_=ext[:])
                wsb[k] = t
            idx_sb = cpool.tile([BLK, w_idx], I16, tag="idx")
            nc.sync.dma_start(out=idx_sb[:], in_=idx_in[:])
            dl_sb = cpool.tile([BLK, tot_s], BF16, tag="dl")
            nc.sync.dma_start(out=dl_sb[:], in_=dl_in[:])

            hT_res = hp.tile([F, NC_N], F32, tag="hres")
            hT_act = hp.tile([F, NC_N], BF16, tag="hact")
            nc.vector.memset(hT_act[:], 0.0)
            arenas_sb = [hp.tile([BLK, ARENA_S, ROW], BF16, tag=f"ar{i}",
                                 name=f"arena{i}")
                         for i in range(NREG)]
            scr = hp.tile([F, BSPLIT], F32, tag="scr")
            stats = hp.tile([F, 8], F32, tag="stats")
            bnsc = hp.tile([F, 8], F32, tag="bnsc")

            for l in (0, 1)[:layer_cap]:
                d = dims[l]
                # ================= phase A: per-node G rows + x_root =======
                for ci in range(n_chunks):
                    c0 = ci * CHUNK
                    cw = min(CHUNK, NC_N - c0)
                    if l == 0:
                        rhs = wp.tile([IN, CHUNK], BF16, tag="xchunk")
                        nc.sync.dma_start(out=rhs[:, :cw], in_=xT[:, c0:c0 + cw])
                        rhs_ap = rhs[:IN, :cw]
                    else:
                        rhs_ap = hT_act[:F, c0:c0 + cw]

                    ps_jm = pp.tile([H * F, CHUNK], F32, tag="jm", space="PSUM")
                    ps_iq = pp.tile([H * F, CHUNK], F32, tag="iq", space="PSUM")
                    ps_r = pp.tile([F, CHUNK], F32, tag="r", space="PSUM")
                    nc.tensor.matmul(out=ps_jm[:, :cw], lhsT=wsb[f"Wn{l}"][:d, :],
                                     rhs=rhs_ap, start=True, stop=True)
                    nc.tensor.matmul(out=ps_iq[:, :cw], lhsT=wsb[f"Wa{l}"][:d, :],
                                     rhs=rhs_ap, start=True, stop=True)
                    nc.tensor.matmul(out=ps_r[:, :cw], lhsT=wsb[f"Wr{l}"][:d, :],
                                     rhs=rhs_ap, start=True, stop=True)
                    nc.vector.tensor_copy(hT_res[:, c0:c0 + cw], ps_r[:, :cw])

                    jm = wp.tile([H * F, CHUNK], BF16, tag="jm_sb")
                    nc.scalar.activation(jm[:, :cw], ps_jm[:, :cw], AF.Identity)
                    # leaky(x) = max(x, 0.2x)
                    lkjm = wp.tile([H * F, CHUNK], BF16, tag="lkjm")
                    nc.scalar.mul(lkjm[:, :cw], ps_jm[:, :cw], LEAKY)
                    nc.vector.tensor_tensor(out=lkjm[:, :cw], in0=lkjm[:, :cw],
                                            in1=jm[:, :cw], op=OP.max)
                    iq = wp.tile([H * F, CHUNK], BF16, tag="iq_sb")
                    nc.scalar.activation(iq[:, :cw], ps_iq[:, :cw], AF.Identity)
                    lkiq = wp.tile([H * F, CHUNK], BF16, tag="lkiq")
                    nc.scalar.mul(lkiq[:, :cw], ps_iq[:, :cw], LEAKY)
                    nc.vector.tensor_tensor(out=lkiq[:, :cw], in0=lkiq[:, :cw],
                                            in1=iq[:, :cw], op=OP.max)
                    ps_s = pp.tile([H, CHUNK], F32, tag="s", space="PSUM")
                    nc.tensor.matmul(out=ps_s[:, :cw], lhsT=wsb[f"avq{l}"][:],
                                     rhs=lkiq[:, :cw], start=True, stop=False)
                    nc.tensor.matmul(out=ps_s[:, :cw], lhsT=wsb[f"avm{l}"][:],
                                     rhs=lkjm[:, :cw], start=False, stop=True)
                    e_sb = wp.tile([H, CHUNK], BF16, tag="esb")
                    nc.scalar.activation(e_sb[:, :cw], ps_s[:, :cw], AF.Exp)
                    # broadcast E over the per-head 64 features via matmul
                    ps_eb = pp.tile([H * F, CHUNK], F32, tag="iq", space="PSUM")
                    nc.tensor.matmul(out=ps_eb[:, :cw], lhsT=wsb["blkones"][:],
                                     rhs=e_sb[:, :cw], start=True, stop=True)
                    eb = wp.tile([H * F, CHUNK], BF16, tag="eb")
                    nc.scalar.activation(eb[:, :cw], ps_eb[:, :cw], AF.Identity)
                    y = wp.tile([H * F, CHUNK], BF16, tag="y")
                    nc.vector.tensor_tensor(out=y[:, :cw], in0=jm[:, :cw],
                                            in1=eb[:, :cw], op=OP.mult)
                    # write G rows (transpose to node-major)
                    for q in range(0, cw, BLK):
                        qw = min(BLK, cw - q)
                        ps_t = pb.tile([BLK, BLK], BF16, tag="tp", space="PSUM")
                        nc.tensor.transpose(out=ps_t[:qw, :], in_=y[:, q:q + qw],
                                            identity=wsb["identb"][:])
                        ps_e = pb.tile([BLK, BLK], BF16, tag="tp", space="PSUM")
                        nc.tensor.transpose(out=ps_e[:qw, :H], in_=e_sb[:, q:q + qw],
                                            identity=wsb["identb"][:H, :H])
                        gt = wp.tile([BLK, ROW], BF16, tag="gt")
                        nc.vector.tensor_copy(gt[:qw, 0:H * F], ps_t[:qw, :])
                        nc.vector.tensor_copy(gt[:qw, H * F:GVAL], ps_e[:qw, :H])
                        nc.sync.dma_start(
                            out=g_src[l][c0 + q:c0 + q + qw, :],
                            in_=gt[:qw, :])
                    # AllGather each region as soon as its rows are written
                    if stage_cap >= 2:
                        for i in range(NREG):
                            if ci == AG_CHUNK[i]:
                                nc.gpsimd.collective_compute(
                                    "AllGather", OP.bypass,
                                    replica_groups=groups,
                                    ins=[g_src[l][R_BOUNDS[i]:R_BOUNDS[i + 1], :]],
                                    outs=[g_fullR[l][i][:]])

                if stage_cap < 2:
                    continue

                # ================= phase B: gather + indicator matmul ======
                if stage_cap < 3:
                    continue
                # Synchronous gather calls are the DEFAULT: the Q7's desc-gen
                # is cheaper in immediate mode (~6.9 vs 8.3 ns/idx) and the
                # DMA overlaps later calls via the 4-queue rotation anyway.
                sync_mode = not os.environ.get("GNN_ASYNC")
                if l == 0:
                    cum_calls = [0, 0, 0, 0]   # per-queue call ordinals
                    prev_prep = [None, None, None, None]
                    prev_trigger = [None, None, None, None]
                    trig_of = {}               # (q, ordinal) -> trigger inst
                    call_ctr = [0]
                emitted = [0] * NREG
                call_trig = {}

                def chain(inst, *prevs):
                    deps = bass.InstructionNameOrderedSet()
                    have = False
                    for pv in prevs:
                        if pv is not None:
                            deps.add(pv.ins.name)
                            have = True
                    if have:
                        inst.ins.add_nosync_dependencies_from(deps)

                def emit_call(reg, k):
                    q = call_ctr[0] % 4
                    call_ctr[0] += 1
                    col0 = meta["col_off"][reg] + k * (CAP // 16)
                    in_view = g_fullR[l][reg][:]
                    arena = arenas_sb[reg]
                    slot0 = (8 * k) % ARENA_S
                    if sync_mode:
                        nc.gpsimd.dma_gather(
                            out_ap=arena[:, slot0:slot0 + 8, :],
                            in_ap=in_view,
                            idxs_ap=idx_sb[:, col0:col0 + CAP // 16],
                            num_idxs=CAP, num_idxs_reg=CAP,
                            elem_size=ROW, queue_num=q)
                        call_trig[(reg, k)] = None
                        return
                    ordinal = cum_calls[q]
                    slot = ordinal % 8
                    gate = None
                    if ordinal >= GATE_D:
                        # ring-capacity gate: call (ordinal-GATE_D) of this
                        # queue must be fully drained -> at most GATE_D calls
                        # (~260 descs/engine of the ring) in flight per
                        # queue, and the 8 sem slots stay unambiguous
                        og = ordinal - GATE_D
                        gate = nc.gpsimd.wait_ge(dma_sems[q][og % 8],
                                                 16 * (og // 8 + 1))
                        chain(gate, trig_of[(q, og)], prev_prep[q])
                    p = nc.gpsimd.dma_gather(
                        out_ap=arena[:, slot0:slot0 + 8, :],
                        in_ap=in_view,
                        idxs_ap=idx_sb[:, col0:col0 + CAP // 16],
                        num_idxs=CAP, num_idxs_reg=CAP,
                        elem_size=ROW, queue_num=q,
                        prepare_only=True, sem=dma_sems[q][slot])
                    nc._gnn_prep_targets[p.ins.name] = 16 * (ordinal // 8 + 1)
                    chain(p, gate, prev_prep[q])
                    prev_prep[q] = p
                    t = nc.gpsimd.trigger_dma(count=1, queue_num=q)
                    chain(t, p, prev_trigger[q])
                    nc._gnn_prep_trig[p.ins.name] = t.ins
                    prev_trigger[q] = t
                    trig_of[(q, ordinal)] = t
                    call_trig[(reg, k)] = t
                    cum_calls[q] = ordinal + 1

                for b in range(nb):
                    bl = blocks[b]
                    b0 = b * BLK
                    bw = min(BLK, NC_N - b0)
                    for i in range(NREG):
                        while emitted[i] < bl["need"][i]:
                            emit_call(i, emitted[i])
                            emitted[i] += 1
                    if stage_cap < 4:
                        continue
                    off = bl["dl_off"]
                    n_sub = bl["n_sub"]
                    ind = wp.tile([BLK, s_max * BLK], BF16, tag="ind", bufs=3)
                    nc.vector.tensor_tensor(
                        out=ind[:, 0:n_sub * BLK].rearrange("p (s i) -> p s i", i=BLK),
                        in0=dl_sb[:, off:off + n_sub][:, :, None]
                            .to_broadcast([BLK, n_sub, BLK]),
                        in1=wsb["iotaw"][:, 0:n_sub * BLK]
                            .rearrange("p (s i) -> p s i", i=BLK),
                        op=OP.is_equal)
                    ps_blk = pb.tile([BLK, GVAL], F32, tag="blk", space="PSUM")
                    for j, (reg, s, e0, e1) in enumerate(bl["subs"]):
                        arena = arenas_sb[reg]
                        mm = nc.tensor.matmul(out=ps_blk[:],
                                              lhsT=ind[:, j * BLK:(j + 1) * BLK],
                                              rhs=arena[:, s % ARENA_S, 0:GVAL],
                                              start=(j == 0), stop=(j == n_sub - 1))
                        tg = call_trig.get((reg, s // 8))
                        if mm is not None and tg is not None:
                            # scheduling-order (no-sync) edge: keep stage
                            # consumers after their call's trigger in the PE
                            # stream, else PE head-of-line blocks on data
                            # whose trigger hasn't dispatched yet
                            deps = bass.InstructionNameOrderedSet()
                            deps.add(tg.ins.name)
                            mm.ins.add_nosync_dependencies_from(deps)
                    sb = wp.tile([BLK, GVAL], F32, tag="sbblk")
                    nc.vector.tensor_copy(sb[:], ps_blk[:])
                    rec = wp.tile([BLK, H], F32, tag="rec")
                    nc.vector.tensor_scalar_add(rec[:], sb[:, H * F:GVAL], 1e-30)
                    nc.vector.reciprocal(rec[:], rec[:])
                    agg = wp.tile([BLK, F], F32, tag="agg")
                    tmp = wp.tile([BLK, F], F32, tag="tmp")
                    nc.scalar.activation(agg[:], sb[:, 0:F], AF.Identity,
                                         scale=rec[:, 0:1])
                    nc.scalar.activation(tmp[:], sb[:, F:2 * F], AF.Identity,
                                         scale=rec[:, 1:2])
                    nc.vector.tensor_add(out=agg[:], in0=agg[:], in1=tmp[:])
                    agg_bf = wp.tile([BLK, F], BF16, tag="aggbf")
                    nc.vector.tensor_copy(agg_bf[:], agg[:])
                    ps_t = pb.tile([BLK, BLK], BF16, tag="tp", space="PSUM")
                    nc.tensor.transpose(out=ps_t[:F, :], in_=agg_bf[:, :F],
                                        identity=wsb["identb"][:])
                    nc.vector.tensor_add(out=hT_res[:, b0:b0 + bw],
                                         in0=hT_res[:, b0:b0 + bw],
                                         in1=ps_t[:F, :bw])

                # ================= BatchNorm + ReLU ========================
                if stage_cap < 5:
                    continue
                # stats in two block-aligned halves so the first can reduce
                # while phase B still works on the second half's blocks
                half = ASPLIT
                nc.vector.reduce_sum(out=stats[:, 0:1], in_=hT_res[:, 0:half],
                                     axis=mybir.AxisListType.X)
                nc.scalar.square(scr[:, 0:half], hT_res[:, 0:half])
                nc.vector.reduce_sum(out=stats[:, 1:2], in_=scr[:, 0:half],
                                     axis=mybir.AxisListType.X)
                nc.vector.reduce_sum(out=stats[:, 4:5],
                                     in_=hT_res[:, half:NC_N],
                                     axis=mybir.AxisListType.X)
                nc.scalar.square(scr[:, 0:NC_N - half], hT_res[:, half:NC_N])
                nc.vector.reduce_sum(out=stats[:, 5:6], in_=scr[:, 0:NC_N - half],
                                     axis=mybir.AxisListType.X)
                nc.vector.tensor_add(out=stats[:, 0:1], in0=stats[:, 0:1],
                                     in1=stats[:, 4:5])
                nc.vector.tensor_add(out=stats[:, 1:2], in0=stats[:, 1:2],
                                     in1=stats[:, 5:6])
                nc.sync.dma_start(out=bn_src[l][:], in_=stats[:, 0:2])
                nc.gpsimd.collective_compute(
                    "AllReduce", OP.add, replica_groups=groups,
                    ins=[bn_src[l][:]], outs=[bn_out[l][:]])
                nc.sync.dma_start(out=stats[:, 2:4], in_=bn_out[l][:])
                nc.scalar.mul(bnsc[:, 0:1], stats[:, 2:3], 1.0 / N)
                nc.scalar.mul(bnsc[:, 1:2], stats[:, 3:4], 1.0 / N)
                nc.vector.tensor_tensor(out=bnsc[:, 2:3], in0=bnsc[:, 0:1],
                                        in1=bnsc[:, 0:1], op=OP.mult)
                nc.vector.tensor_tensor(out=bnsc[:, 2:3], in0=bnsc[:, 1:2],
                                        in1=bnsc[:, 2:3], op=OP.subtract)
                nc.vector.tensor_scalar_add(bnsc[:, 2:3], bnsc[:, 2:3], BN_EPS)
                nc.vector.reciprocal(bnsc[:, 3:4], bnsc[:, 2:3])
                nc.scalar.sqrt(bnsc[:, 4:5], bnsc[:, 3:4])
                nc.vector.tensor_tensor(out=bnsc[:, 5:6], in0=bnsc[:, 4:5],
                                        in1=wsb[f"bn{l}"][:, 0:1], op=OP.mult)
                nc.vector.tensor_tensor(out=bnsc[:, 6:7], in0=bnsc[:, 0:1],
                                        in1=bnsc[:, 5:6], op=OP.mult)
                nc.vector.tensor_tensor(out=bnsc[:, 6:7], in0=wsb[f"bn{l}"][:, 1:2],
                                        in1=bnsc[:, 6:7], op=OP.subtract)
                # apply per chunk so the next layer / head can start on
                # early chunks while later ones are still being written
                for ci in range(n_chunks):
                    c0 = ci * CHUNK
                    cw = min(CHUNK, NC_N - c0)
                    nc.scalar.activation(hT_act[:, c0:c0 + cw],
                                         hT_res[:, c0:c0 + cw],
                                         AF.Relu, bias=bnsc[:, 6:7],
                                         scale=bnsc[:, 5:6])

            # ================= head ========================================
            for ci in range(n_chunks):
                c0 = ci * CHUNK
                cw = min(CHUNK, NC_N - c0)
                ps_o = pp.tile([3, CHUNK], F32, tag="s", space="PSUM")
                nc.tensor.matmul(out=ps_o[:, :cw], lhsT=wsb["headW"][:],
                                 rhs=hT_act[:F, c0:c0 + cw], start=True, stop=True)
                osb = wp.tile([3, CHUNK], F32, tag="osb")
                nc.scalar.activation(osb[:, :cw], ps_o[:, :cw],
                                     AF.Identity, bias=wsb["headb"][:, 0:1])
                nc.sync.dma_start(out=out_ext[:, c0:c0 + cw], in_=osb[:, :cw])

    return nc


# ---------------------------------------------------------------- run cache
_CACHE = {}


def _build_inputs(inputs, meta, idx_full, dl_dev):
    w = pack_weights(inputs, meta["s_max"])
    x = np.asarray(inputs["x"], np.float32)
    in_maps = []
    for c in range(N_CORES):
        m = dict(w)
        m["xT"] = np.ascontiguousarray(
            x[c * NC_N:(c + 1) * NC_N, :].T).astype(BF)
        m["idx"] = np.ascontiguousarray(idx_full[c])
        m["dstloc"] = np.ascontiguousarray(dl_dev[c])
        in_maps.append(m)
    return in_maps


def kernel(**inputs):
    from concourse.bass_utils import run_bass_kernel_spmd

    _install_hookshim()
    edge = np.asarray(inputs["edge_index"])
    key = hashlib.sha1(edge.tobytes()).hexdigest()
    if key not in _CACHE:
        idx_full, dl_dev, meta = preprocess(edge)
        nc = build_program(meta)
        nc.finalize()
        if os.environ.get("GNN_ASYNC"):
            n_remap, n_del, n_xfer = remap_dmasw_waits(nc)
            print(f"remapped DMASW waits on {n_remap} insts, deleted "
                  f"{n_del} IncSwdgeSem, moved {n_xfer} waits to triggers")
        n_fix = legalize_waits(nc)
        if n_fix:
            print(f"legalize_waits fixed {n_fix} instructions post-finalize")
        _CACHE[key] = (idx_full, dl_dev, meta, nc)
    idx_full, dl_dev, meta, nc = _CACHE[key]
    in_maps = _build_inputs(inputs, meta, idx_full, dl_dev)
    res = run_bass_kernel_spmd(
        nc, in_maps, list(range(N_CORES)),
        trace=bool(os.environ.get("GNN_TRACE")))
    if res.exec_time_ns is not None:
        print(f"HW exec time: {res.exec_time_ns} ns")
    out = np.concatenate([res.results[c]["out"] for c in range(N_CORES)],
                         axis=1)  # [3, N]
    return np.ascontiguousarray(out.T).astype(np.float32)


# revision 78
# speedup vs baseline: 1.2284x; 1.0130x over previous
"""AttnGraphSAGE on 8 Trainium2 NeuronCores (Bass/Tile) — v2.

Math restructuring (unchanged from v1): attention logits depend only on the
SOURCE node, so the whole edge phase is ONE segment-sum over dst of per-src
rows G[n] = [E_0*x_jm_0 (64) | E_1*x_jm_1 (64) | E_0 | E_1] (130 values).

v2 performance changes (2268us -> ~1274us on 8 cores):
  * G rows are bf16, 256-elem / 512B strides (was f32 768B): halves the
    random-gather HBM traffic and the AllGather volume.  All matmul operands
    (weights, activations, indicator) are bf16 -> 1-pass PE instead of 4.
  * The G table is AllGather'd in FOUR per-core row regions; each region's
    collective is issued as soon as phase A finishes its rows, so early
    regions' gathers overlap both the rest of phase A and the later
    AllGathers.  Every region stays < 32768 total rows, so region base
    addresses double as the int16-index split.
  * Gather calls are PACKED to exactly 1024 indices (the Q7 per-call
    ceiling) spanning dst-block boundaries; each call fills 8 consecutive
    subtiles of a 32-subtile ring arena per region.  A block's indicator
    matmuls consume the subtiles it touches; boundary subtiles are consumed
    by both adjacent blocks with foreign slots killed by dstloc=-1.  This
    minimizes Q7 descriptor-generation calls (~7ns/idx on the critical
    engine) with zero padding waste.
  * Synchronous (immediate) gather calls: measured cheaper per idx than the
    PREPARE_ONLY+trigger path, and the 4-queue rotation overlaps the DMA
    with subsequent descriptor generation anyway.  (GNN_ASYNC=1 selects the
    prepare/trigger path, kept for experiments.)
  * PSUM->bf16 casts and the per-head 1/denom scaling run on the Scalar
    engine (per-partition scale operand), and the is_equal indicator build
    compares against a materialized iota operand, unloading the DVE which
    is co-critical with the Q7/DMA during the edge phase.
  * BN stats reduce in block-aligned halves (first half starts during the
    phase-B tail) and the BN ReLU applies per chunk so the next layer's
    matmuls start on early chunks.
  * Per-core counts padded only to the max across the 8 cores so the
    program stays SPMD-uniform; 0-padded (no trailing -1 indices).
"""
import os
import sys
import types
import hashlib
import contextlib

sys.path.insert(0, "/opt/trn_rl_repo")

import numpy as np
import ml_dtypes

import concourse.bass as bass
import concourse.bacc as bacc
import concourse.mybir as mybir
from concourse import tile

# ---------------------------------------------------------------- constants
N = 50000
E = 800000
IN = 128
F = 64
H = 2
N_CORES = 8
NC_N = N // N_CORES          # 6250 nodes per core
BLK = 128                    # dst nodes per block
ROW = 256                    # G row stride in bf16 elems (512B)
GVAL = 2 * F + H             # 130 used cols
CHUNK = 512                  # phase-A node chunk
# G-table AllGather regions (per-core row ranges).  4 regions so the first
# regions' gathers start while later regions are still being computed /
# AllGather'd; each region stays < 32768 total rows for int16 indices.
R_BOUNDS = [0, 1536, 3072, 4608, NC_N]
NREG = len(R_BOUNDS) - 1
R_SIZE = [R_BOUNDS[i + 1] - R_BOUNDS[i] for i in range(NREG)]
AG_CHUNK = [(R_BOUNDS[i + 1] + CHUNK - 1) // CHUNK - 1 for i in range(NREG)]
ASPLIT = 3072                # BN stats half split (block-aligned)
BSPLIT = NC_N - ASPLIT
CAP = int(os.environ.get("GNN_CAP", "1024"))   # idxs per gather call (HW max)
ARENA_S = 32                 # ring-arena subtiles per region (4 calls)
GATE_D = 4                   # calls in flight per queue (ring + sem-slot cap)
F32 = mybir.dt.float32
BF16 = mybir.dt.bfloat16
FP8 = mybir.dt.float8e4
I16 = mybir.dt.int16
AF = mybir.ActivationFunctionType
OP = mybir.AluOpType
BN_EPS = 1e-5
LEAKY = 0.2
BF = ml_dtypes.bfloat16


# ------------------------------------------------------- axon profile shim
def _install_hookshim():
    if "antenv.axon_hooks" in sys.modules:
        return
    mod = types.ModuleType("antenv.axon_hooks")
    _h = [None]
    mod.set_axon_ntff_profile_hook = lambda h: _h.__setitem__(0, h)
    mod.get_axon_ntff_profile_hook = lambda: _h[0]
    try:
        import antenv
        sys.modules["antenv.axon_hooks"] = mod
        antenv.axon_hooks = mod
        from trn_agent_boot.trn_boot import _ntff_profile_via_ctypes
        mod.set_axon_ntff_profile_hook(
            _ntff_profile_via_ctypes("/opt/axon/libaxon_pjrt.so")
        )
    except Exception:
        pass


def remap_dmasw_waits(nc):
    """Remap waits on Tile's DMASW lane semaphores to the per-queue gather
    DMA-completion sems.

    Tile assigned each PREPARE_ONLY gather prep a DMASW lane (round-robin)
    and derived all downstream waits (consumers, ring flow control) as
    ``DMASW{lane} >= 16*tick``.  But the sem actually baked into the
    descriptors (and bumped by the SDMA engines) is our per-queue gdma sem,
    so those lane sems never move.  Each prep records its assigned
    (lane proc, tick); since each queue's ring is FIFO, the k-th prep of
    queue q has completed exactly when gdma{q} >= 16*k.  Rewrite every
    DMASW wait for (lane, tick) into the equivalent (and race-free)
    per-queue wait."""
    from concourse.tile_sem_assignment import PROC_NAME_TO_IDX
    inv_proc = {v: k for k, v in PROC_NAME_TO_IDX.items()}

    insts = []
    for func in nc.m.functions:
        for block in func.blocks:
            insts.extend(block.instructions)

    # (lane_name, 16*tick) -> (gdma sem id, gdma name, block-level target)
    lane_map = {}
    for inst in insts:
        if type(inst).__name__ == "InstDMAGatherAnt" and \
                getattr(inst, "gen_mode", 0) == 1:
            lane = inv_proc[inst.bass_scheduled_proc]
            upd = inst.sync_info.on_update[0]
            assert upd.ant_name.startswith("gdma"), upd.ant_name
            key = (lane, 16 * inst.bass_scheduled_tick)
            assert key not in lane_map, key
            lane_map[key] = (upd.id, upd.ant_name,
                             nc._gnn_prep_targets[inst.name])

    # waits with these prefixes are deferred from a prep to its trigger:
    # the prep only writes ring descriptors; the DMA (which actually touches
    # the arena / g_full) fires at the trigger, so enforcing reader-WAR and
    # collective deps there frees desc-gen to run ahead.
    XFER = ("PE_", "DVE_", "Act", "Collectives_")
    n = 0
    n_del = 0
    n_xfer = 0
    for func in nc.m.functions:
        for block in func.blocks:
            kept = []
            for inst in block.instructions:
                # Tile's per-prep DMASW shadow-sem maintenance is dead weight
                # once nothing uses the lane sems (1.65us of Pool each, plus
                # serializing ring-drain waits); the ring-capacity gates keep
                # the ring below capacity without it.
                if type(inst).__name__ == "InstIncSwdgeSem":
                    n_del += 1
                    continue
                kept.append(inst)
                si = inst.sync_info
                if not (si and si.on_wait):
                    continue
                changed = False
                new_waits = []
                trig = nc._gnn_prep_trig.get(inst.name)
                for w in si.on_wait:
                    if w.ant_name and w.ant_name.startswith("DMASW"):
                        lane = w.ant_name.split("_")[0]
                        sid, sname, thresh = lane_map[(lane, w.wait_value)]
                        new_waits.append(mybir.SyncWait(
                            sync_type="semaphore", id=sid,
                            wait_mode="sem-ge-imm",
                            wait_value=thresh, ant_name=sname))
                        changed = True
                    elif trig is not None and w.ant_name and \
                            w.ant_name.startswith(XFER):
                        tsi = trig.sync_info
                        tsi.on_wait = list(tsi.on_wait or []) + [w]
                        changed = True
                        n_xfer += 1
                    else:
                        new_waits.append(w)
                if changed:
                    si.on_wait = new_waits
                    n += 1
            block.instructions[:] = kept
    return n, n_del, n_xfer


# ------------------------------------------------------------ wait legalize
def legalize_waits(nc):
    """TRN2 TPB instructions have ONE sync-wait slot (EventSemaphore has 2);
    hoist extra waits left by the Tile scheduler into EVSEM prequels."""
    n_fixed = 0
    for func in nc.m.functions:
        for block in func.blocks:
            new_insts = []
            for inst in block.instructions:
                si = inst.sync_info
                waits = list(si.on_wait) if si and si.on_wait else []
                cap = 2 if isinstance(inst, mybir.InstEventSemaphore) else 1
                if isinstance(inst, mybir.InstDrain):
                    cap = 1
                if len(waits) > cap:
                    extra, keep = waits[:-cap], waits[-cap:]
                    for i in range(0, len(extra), 2):
                        new_insts.append(
                            mybir.InstEventSemaphore(
                                name=nc.get_next_instruction_name(),
                                ins=[],
                                outs=[],
                                engine=inst.engine,
                                sync_info=mybir.SyncInfo(
                                    on_wait=extra[i:i + 2], on_update=[]
                                ),
                            )
                        )
                    si.on_wait = keep
                    n_fixed += 1
                new_insts.append(inst)
            block.instructions[:] = new_insts
    return n_fixed


# ----------------------------------------------------------- host preprocess
def preprocess(edge_index):
    """Sort edges by dst, partition per core / per 128-dst block, split each
    block's edges into A/B-region runs (by source row within its owner core),
    pad counts to the per-block max across cores (program is SPMD-uniform).

    Each region's padded edge stream is then PACKED into gather calls of
    exactly CAP indices spanning block boundaries (the Q7 per-call fixed
    cost ~4us dominates, so call count is what matters).  Calls write 8
    consecutive subtiles of a 32-subtile ring arena per region; a block's
    indicator matmul consumes the (possibly boundary-shared) subtiles it
    touches, with foreign slots killed by dl=-1."""
    nb = (NC_N + BLK - 1) // BLK
    src = np.asarray(edge_index[0], np.int64)
    dst = np.asarray(edge_index[1], np.int64)
    order = np.argsort(dst, kind="stable")
    ds, ss = dst[order], src[order]

    core = ds // NC_N
    blk = (ds - core * NC_N) // BLK
    gblk = core * nb + blk
    n_gblk = N_CORES * nb
    bbounds = np.searchsorted(gblk, np.arange(n_gblk + 1))

    # source slot within the AllGather'd table regions
    sc = ss // NC_N
    r = ss - sc * NC_N
    ri = np.searchsorted(np.asarray(R_BOUNDS), r, side="right") - 1
    base_arr = np.asarray([R_BOUNDS[i] for i in range(NREG)])
    size_arr = np.asarray(R_SIZE)
    slot = sc * size_arr[ri] + (r - base_arr[ri])

    runs = {}    # (core, block, region) -> (slots, dls)
    n_r = np.zeros((NREG, N_CORES, nb), np.int64)
    for g in range(n_gblk):
        e0, e1 = bbounds[g], bbounds[g + 1]
        c, b = g // nb, g % nb
        base = c * NC_N + b * BLK
        sl, dl, rr = slot[e0:e1], ds[e0:e1] - base, ri[e0:e1]
        for i in range(NREG):
            m = rr == i
            s_i, d_i = sl[m], dl[m]
            # ascending slot order -> ascending HBM addresses
            o = np.argsort(s_i, kind="stable")
            runs[(c, b, i)] = (s_i[o], d_i[o])
            n_r[i, c, b] = len(s_i)

    n_u = n_r.max(axis=1).astype(int)     # [NREG, nb] uniform counts

    # region stream layout: block b's run occupies [start[b], start[b]+n)
    starts = []
    pads = []
    ncalls = []
    for i in range(NREG):
        st = np.concatenate([[0], np.cumsum(n_u[i])])
        total_pad = (int(st[-1]) + CAP - 1) // CAP * CAP
        starts.append(st)
        pads.append(total_pad)
        ncalls.append(total_pad // CAP)

    # per block: touched subtiles per region + dl columns
    blocks = []
    tot_s = 0
    for b in range(nb):
        entry = dict(dl_off=tot_s, subs=[], need=[0] * NREG)
        for i in range(NREG):
            e0, e1 = int(starts[i][b]), int(starts[i][b] + n_u[i][b])
            for s in range(e0 // BLK, (e1 + BLK - 1) // BLK):
                entry["subs"].append((i, s, e0, e1))
            entry["need"][i] = (e1 + CAP - 1) // CAP if e1 > 0 else 0
        entry["n_sub"] = len(entry["subs"])
        tot_s += entry["n_sub"]
        blocks.append(entry)

    # index planes: region streams wrapped per call (CAP idx = CAP//16 cols)
    col_off = [0]
    for i in range(NREG):
        col_off.append(col_off[-1] + ncalls[i] * (CAP // 16))
    w_idx = col_off[-1]
    idx_dev = np.zeros((N_CORES, 16, w_idx), np.int16)
    dl_dev = np.full((N_CORES, BLK, tot_s), -1.0, np.float32)

    for c in range(N_CORES):
        for i in range(NREG):
            streamv = np.zeros((pads[i],), np.int64)
            for b in range(nb):
                v, _ = runs[(c, b, i)]
                e0 = int(starts[i][b])
                streamv[e0:e0 + len(v)] = v
            # wrap16 whole region stream: idx k -> (p=k%16, col=k//16)
            idx_dev[c, :, col_off[i]:col_off[i] + pads[i] // 16] = \
                streamv.reshape(-1, 16).T.astype(np.int16)
        for b in range(nb):
            bl = blocks[b]
            for k, (i, s, e0, e1) in enumerate(bl["subs"]):
                _, dvals = runs[(c, b, i)]
                col = np.full((BLK,), -1.0, np.float32)
                lo = max(e0, s * BLK)
                hi = min(e0 + len(dvals), (s + 1) * BLK)
                if hi > lo:
                    col[lo - s * BLK:hi - s * BLK] = dvals[lo - e0:hi - e0]
                dl_dev[c, :, bl["dl_off"] + k] = col

    idx_full = np.tile(idx_dev, (1, 8, 1))     # replicate to 128 partitions
    s_max = max(bl["n_sub"] for bl in blocks)
    meta = dict(nb=nb, blocks=blocks, w_idx=w_idx, tot_s=tot_s, s_max=s_max,
                ncalls=ncalls, col_off=col_off)
    return idx_full, dl_dev.astype(BF), meta


def pack_weights(inp, s_max):
    """Host-side packing of the small replicated weight tensors (bf16)."""
    def bd(av):  # [H, 2F] -> block-diag [H*F, H] halves (query, msg)
        av = np.asarray(av, np.float32)
        q = np.zeros((H * F, H), np.float32)
        m = np.zeros((H * F, H), np.float32)
        for h in range(H):
            q[h * F:(h + 1) * F, h] = av[h, :F]
            m[h * F:(h + 1) * F, h] = av[h, F:]
        return q, m

    w = {}
    for l in (0, 1):
        w[f"Wr{l}"] = np.asarray(inp[f"Wr{l}"], np.float32).astype(BF)
        w[f"Wn{l}"] = np.asarray(inp[f"Wn{l}"], np.float32).astype(BF)
        w[f"Wa{l}"] = np.asarray(inp[f"Wa{l}"], np.float32).astype(BF)
        q_, m_ = bd(inp[f"av{l}"])
        w[f"avq{l}"], w[f"avm{l}"] = q_.astype(BF), m_.astype(BF)
        w[f"bn{l}"] = np.stack(
            [np.asarray(inp[f"g{l}"], np.float32),
             np.asarray(inp[f"b{l}"], np.float32)], axis=1)  # [64,2] f32
    w["headW"] = np.asarray(inp["head_W"], np.float32).astype(BF)
    w["headb"] = np.asarray(inp["head_b"], np.float32).reshape(3, 1)
    w["iota"] = np.broadcast_to(np.arange(BLK, dtype=np.float32),
                                (BLK, BLK)).astype(BF)
    w["iotaw"] = np.broadcast_to(
        np.tile(np.arange(BLK, dtype=np.float32), s_max),
        (BLK, s_max * BLK)).astype(BF)
    w["identb"] = np.eye(BLK, dtype=np.float32).astype(BF)
    w["identf"] = np.eye(BLK, dtype=np.float32)
    bo = np.zeros((H, H * F), np.float32)
    for h in range(H):
        bo[h, h * F:(h + 1) * F] = 1.0
    w["blkones"] = bo.astype(BF)
    return w


# ------------------------------------------------------------ device program
def build_program(meta):
    nb = meta["nb"]
    blocks = meta["blocks"]
    w_idx = meta["w_idx"]
    tot_s = meta["tot_s"]
    s_max = meta["s_max"]
    dims = [IN, F]

    nc = bacc.Bacc(None, num_swdge_queues=4)
    nc._gnn_prep_targets = {}   # prep inst name -> completion sem target
    nc._gnn_prep_trig = {}      # prep inst name -> its trigger (mybir inst)

    # ---- I/O
    xT = nc.declare_dram_parameter("xT", [IN, NC_N], BF16, isOutput=False)
    idx_in = nc.declare_dram_parameter("idx", [BLK, w_idx], I16, isOutput=False)
    dl_in = nc.declare_dram_parameter("dstloc", [BLK, tot_s], BF16, isOutput=False)
    wext = {}
    for l in (0, 1):
        d = dims[l]
        wext[f"Wr{l}"] = nc.declare_dram_parameter(f"Wr{l}", [d, F], BF16, isOutput=False)
        wext[f"Wn{l}"] = nc.declare_dram_parameter(f"Wn{l}", [d, H * F], BF16, isOutput=False)
        wext[f"Wa{l}"] = nc.declare_dram_parameter(f"Wa{l}", [d, H * F], BF16, isOutput=False)
        wext[f"avq{l}"] = nc.declare_dram_parameter(f"avq{l}", [H * F, H], BF16, isOutput=False)
        wext[f"avm{l}"] = nc.declare_dram_parameter(f"avm{l}", [H * F, H], BF16, isOutput=False)
        wext[f"bn{l}"] = nc.declare_dram_parameter(f"bn{l}", [F, 2], F32, isOutput=False)
    wext["headW"] = nc.declare_dram_parameter("headW", [F, 3], BF16, isOutput=False)
    wext["headb"] = nc.declare_dram_parameter("headb", [3, 1], F32, isOutput=False)
    wext["iota"] = nc.declare_dram_parameter("iota", [BLK, BLK], BF16, isOutput=False)
    wext["iotaw"] = nc.declare_dram_parameter("iotaw", [BLK, s_max * BLK], BF16, isOutput=False)
    wext["identb"] = nc.declare_dram_parameter("identb", [BLK, BLK], BF16, isOutput=False)
    wext["identf"] = nc.declare_dram_parameter("identf", [BLK, BLK], F32, isOutput=False)
    wext["blkones"] = nc.declare_dram_parameter("blkones", [H, H * F], BF16, isOutput=False)
    out_ext = nc.declare_dram_parameter("out", [3, NC_N], F32, isOutput=True)

    # ---- internal DRAM
    g_src = [nc.dram_tensor(f"g_src{l}", [NC_N, ROW], BF16) for l in (0, 1)]
    g_fullR = [[nc.dram_tensor(f"g_fullR{i}_{l}", [N_CORES * R_SIZE[i], ROW],
                               BF16, addr_space="Shared")
                for i in range(NREG)] for l in (0, 1)]
    bn_src = [nc.dram_tensor(f"bn_src{l}", [F, 2], F32) for l in (0, 1)]
    bn_out = [nc.dram_tensor(f"bn_out{l}", [F, 2], F32, addr_space="Shared")
              for l in (0, 1)]
    groups = [list(range(N_CORES))]

    n_chunks = (NC_N + CHUNK - 1) // CHUNK
    stage_cap = int(os.environ.get("GNN_STAGE", "9"))
    layer_cap = int(os.environ.get("GNN_LAYERS", "2"))

    # 8 rotating completion sems per queue: call k (per-queue ordinal) uses
    # slot k%8.  With <=8 calls in flight per queue (ring gates), each slot
    # has at most one call in flight, so the threshold 16*(k//8+1) is exact.
    dma_sems = [[nc.alloc_semaphore(f"gdma{q}_{j}") for j in range(8)]
                for q in range(4)]

    with tile.TileContext(nc) as tc:
        with contextlib.ExitStack() as ctx:
            cpool = ctx.enter_context(tc.tile_pool(name="const", bufs=1))
            wp = ctx.enter_context(tc.tile_pool(name="work", bufs=2))
            hp = ctx.enter_context(tc.tile_pool(name="resid", bufs=1))
            pp = ctx.enter_context(tc.tile_pool(name="psA", bufs=1, space="PSUM"))
            pb = ctx.enter_context(tc.tile_pool(name="psB", bufs=2, space="PSUM"))

            # ---- load constants
            wsb = {}
            for k, ext in wext.items():
                t = cpool.tile(list(ext.shape), ext.dtype, tag=k)
                nc.sync.dma_start(out=t[:], in_=ext[:])
                wsb[k] = t
            idx_sb = cpool.tile([BLK, w_idx], I16, tag="idx")
            nc.sync.dma_start(out=idx_sb[:], in_=idx_in[:])
            dl_sb = cpool.tile([BLK, tot_s], BF16, tag="dl")
            nc.sync.dma_start(out=dl_sb[:], in_=dl_in[:])

            hT_res = hp.tile([F, NC_N], F32, tag="hres")
            hT_act = hp.tile([F, NC_N], BF16, tag="hact")
            nc.vector.memset(hT_act[:], 0.0)
            arenas_sb = [hp.tile([BLK, ARENA_S, ROW], BF16, tag=f"ar{i}",
                                 name=f"arena{i}")
                         for i in range(NREG)]
            scr = hp.tile([F, BSPLIT], F32, tag="scr")
            stats = hp.tile([F, 8], F32, tag="stats")
            bnsc = hp.tile([F, 8], F32, tag="bnsc")

            for l in (0, 1)[:layer_cap]:
                d = dims[l]
                # ================= phase A: per-node G rows + x_root =======
                for ci in range(n_chunks):
                    c0 = ci * CHUNK
                    cw = min(CHUNK, NC_N - c0)
                    if l == 0:
                        rhs = wp.tile([IN, CHUNK], BF16, tag="xchunk")
                        nc.sync.dma_start(out=rhs[:, :cw], in_=xT[:, c0:c0 + cw])
                        rhs_ap = rhs[:IN, :cw]
                    else:
                        rhs_ap = hT_act[:F, c0:c0 + cw]

                    ps_jm = pp.tile([H * F, CHUNK], F32, tag="jm", space="PSUM")
                    ps_iq = pp.tile([H * F, CHUNK], F32, tag="iq", space="PSUM")
                    ps_r = pp.tile([F, CHUNK], F32, tag="r", space="PSUM")
                    nc.tensor.matmul(out=ps_jm[:, :cw], lhsT=wsb[f"Wn{l}"][:d, :],
                                     rhs=rhs_ap, start=True, stop=True)
                    nc.tensor.matmul(out=ps_iq[:, :cw], lhsT=wsb[f"Wa{l}"][:d, :],
                                     rhs=rhs_ap, start=True, stop=True)
                    nc.tensor.matmul(out=ps_r[:, :cw], lhsT=wsb[f"Wr{l}"][:d, :],
                                     rhs=rhs_ap, start=True, stop=True)
                    nc.vector.tensor_copy(hT_res[:, c0:c0 + cw], ps_r[:, :cw])

                    jm = wp.tile([H * F, CHUNK], BF16, tag="jm_sb")
                    nc.scalar.activation(jm[:, :cw], ps_jm[:, :cw], AF.Identity)
                    # leaky(x) = max(x, 0.2x)
                    lkjm = wp.tile([H * F, CHUNK], BF16, tag="lkjm")
                    nc.scalar.mul(lkjm[:, :cw], ps_jm[:, :cw], LEAKY)
                    nc.vector.tensor_tensor(out=lkjm[:, :cw], in0=lkjm[:, :cw],
                                            in1=jm[:, :cw], op=OP.max)
                    iq = wp.tile([H * F, CHUNK], BF16, tag="iq_sb")
                    nc.scalar.activation(iq[:, :cw], ps_iq[:, :cw], AF.Identity)
                    lkiq = wp.tile([H * F, CHUNK], BF16, tag="lkiq")
                    nc.scalar.mul(lkiq[:, :cw], ps_iq[:, :cw], LEAKY)
                    nc.vector.tensor_tensor(out=lkiq[:, :cw], in0=lkiq[:, :cw],
                                            in1=iq[:, :cw], op=OP.max)
                    ps_s = pp.tile([H, CHUNK], F32, tag="s", space="PSUM")
                    nc.tensor.matmul(out=ps_s[:, :cw], lhsT=wsb[f"avq{l}"][:],
                                     rhs=lkiq[:, :cw], start=True, stop=False)
                    nc.tensor.matmul(out=ps_s[:, :cw], lhsT=wsb[f"avm{l}"][:],
                                     rhs=lkjm[:, :cw], start=False, stop=True)
                    e_sb = wp.tile([H, CHUNK], BF16, tag="esb")
                    nc.scalar.activation(e_sb[:, :cw], ps_s[:, :cw], AF.Exp)
                    # broadcast E over the per-head 64 features via matmul
                    ps_eb = pp.tile([H * F, CHUNK], F32, tag="iq", space="PSUM")
                    nc.tensor.matmul(out=ps_eb[:, :cw], lhsT=wsb["blkones"][:],
                                     rhs=e_sb[:, :cw], start=True, stop=True)
                    eb = wp.tile([H * F, CHUNK], BF16, tag="eb")
                    nc.scalar.activation(eb[:, :cw], ps_eb[:, :cw], AF.Identity)
                    y = wp.tile([H * F, CHUNK], BF16, tag="y")
                    nc.vector.tensor_tensor(out=y[:, :cw], in0=jm[:, :cw],
                                            in1=eb[:, :cw], op=OP.mult)
                    # write G rows (transpose to node-major)
                    for q in range(0, cw, BLK):
                        qw = min(BLK, cw - q)
                        ps_t = pb.tile([BLK, BLK], BF16, tag="tp", space="PSUM")
                        nc.tensor.transpose(out=ps_t[:qw, :], in_=y[:, q:q + qw],
                                            identity=wsb["identb"][:])
                        ps_e = pb.tile([BLK, BLK], BF16, tag="tp", space="PSUM")
                        nc.tensor.transpose(out=ps_e[:qw, :H], in_=e_sb[:, q:q + qw],
                                            identity=wsb["identb"][:H, :H])
                        gt = wp.tile([BLK, ROW], BF16, tag="gt")
                        nc.vector.tensor_copy(gt[:qw, 0:H * F], ps_t[:qw, :])
                        nc.vector.tensor_copy(gt[:qw, H * F:GVAL], ps_e[:qw, :H])
                        nc.sync.dma_start(
                            out=g_src[l][c0 + q:c0 + q + qw, :],
                            in_=gt[:qw, :])
                    # AllGather each region as soon as its rows are written
                    if stage_cap >= 2:
                        for i in range(NREG):
                            if ci == AG_CHUNK[i]:
                                nc.gpsimd.collective_compute(
                                    "AllGather", OP.bypass,
                                    replica_groups=groups,
                                    ins=[g_src[l][R_BOUNDS[i]:R_BOUNDS[i + 1], :]],
                                    outs=[g_fullR[l][i][:]])

                if stage_cap < 2:
                    continue

                # ================= phase B: gather + indicator matmul ======
                if stage_cap < 3:
                    continue
                # Synchronous gather calls are the DEFAULT: the Q7's desc-gen
                # is cheaper in immediate mode (~6.9 vs 8.3 ns/idx) and the
                # DMA overlaps later calls via the 4-queue rotation anyway.
                sync_mode = not os.environ.get("GNN_ASYNC")
                if l == 0:
                    cum_calls = [0, 0, 0, 0]   # per-queue call ordinals
                    prev_prep = [None, None, None, None]
                    prev_trigger = [None, None, None, None]
                    trig_of = {}               # (q, ordinal) -> trigger inst
                    call_ctr = [0]
                emitted = [0] * NREG
                call_trig = {}

                def chain(inst, *prevs):
                    deps = bass.InstructionNameOrderedSet()
                    have = False
                    for pv in prevs:
                        if pv is not None:
                            deps.add(pv.ins.name)
                            have = True
                    if have:
                        inst.ins.add_nosync_dependencies_from(deps)

                def emit_call(reg, k):
                    q = call_ctr[0] % 4
                    call_ctr[0] += 1
                    col0 = meta["col_off"][reg] + k * (CAP // 16)
                    in_view = g_fullR[l][reg][:]
                    arena = arenas_sb[reg]
                    slot0 = (8 * k) % ARENA_S
                    if sync_mode:
                        nc.gpsimd.dma_gather(
                            out_ap=arena[:, slot0:slot0 + 8, :],
                            in_ap=in_view,
                            idxs_ap=idx_sb[:, col0:col0 + CAP // 16],
                            num_idxs=CAP, num_idxs_reg=CAP,
                            elem_size=ROW, queue_num=q)
                        call_trig[(reg, k)] = None
                        return
                    ordinal = cum_calls[q]
                    slot = ordinal % 8
                    gate = None
                    if ordinal >= GATE_D:
                        # ring-capacity gate: call (ordinal-GATE_D) of this
                        # queue must be fully drained -> at most GATE_D calls
                        # (~260 descs/engine of the ring) in flight per
                        # queue, and the 8 sem slots stay unambiguous
                        og = ordinal - GATE_D
                        gate = nc.gpsimd.wait_ge(dma_sems[q][og % 8],
                                                 16 * (og // 8 + 1))
                        chain(gate, trig_of[(q, og)], prev_prep[q])
                    p = nc.gpsimd.dma_gather(
                        out_ap=arena[:, slot0:slot0 + 8, :],
                        in_ap=in_view,
                        idxs_ap=idx_sb[:, col0:col0 + CAP // 16],
                        num_idxs=CAP, num_idxs_reg=CAP,
                        elem_size=ROW, queue_num=q,
                        prepare_only=True, sem=dma_sems[q][slot])
                    nc._gnn_prep_targets[p.ins.name] = 16 * (ordinal // 8 + 1)
                    chain(p, gate, prev_prep[q])
                    prev_prep[q] = p
                    t = nc.gpsimd.trigger_dma(count=1, queue_num=q)
                    chain(t, p, prev_trigger[q])
                    nc._gnn_prep_trig[p.ins.name] = t.ins
                    prev_trigger[q] = t
                    trig_of[(q, ordinal)] = t
                    call_trig[(reg, k)] = t
                    cum_calls[q] = ordinal + 1

                for b in range(nb):
                    bl = blocks[b]
                    b0 = b * BLK
                    bw = min(BLK, NC_N - b0)
                    for i in range(NREG):
                        while emitted[i] < bl["need"][i]:
                            emit_call(i, emitted[i])
                            emitted[i] += 1
                    if stage_cap < 4:
                        continue
                    off = bl["dl_off"]
                    n_sub = bl["n_sub"]
                    ind = wp.tile([BLK, s_max * BLK], BF16, tag="ind", bufs=3)
                    nc.vector.tensor_tensor(
                        out=ind[:, 0:n_sub * BLK].rearrange("p (s i) -> p s i", i=BLK),
                        in0=dl_sb[:, off:off + n_sub][:, :, None]
                            .to_broadcast([BLK, n_sub, BLK]),
                        in1=wsb["iotaw"][:, 0:n_sub * BLK]
                            .rearrange("p (s i) -> p s i", i=BLK),
                        op=OP.is_equal)
                    ps_blk = pb.tile([BLK, GVAL], F32, tag="blk", space="PSUM")
                    for j, (reg, s, e0, e1) in enumerate(bl["subs"]):
                        arena = arenas_sb[reg]
                        mm = nc.tensor.matmul(out=ps_blk[:],
                                              lhsT=ind[:, j * BLK:(j + 1) * BLK],
                                              rhs=arena[:, s % ARENA_S, 0:GVAL],
                                              start=(j == 0), stop=(j == n_sub - 1))
                        tg = call_trig.get((reg, s // 8))
                        if mm is not None and tg is not None:
                            # scheduling-order (no-sync) edge: keep stage
                            # consumers after their call's trigger in the PE
                            # stream, else PE head-of-line blocks on data
                            # whose trigger hasn't dispatched yet
                            deps = bass.InstructionNameOrderedSet()
                            deps.add(tg.ins.name)
                            mm.ins.add_nosync_dependencies_from(deps)
                    sb = wp.tile([BLK, GVAL], F32, tag="sbblk")
                    nc.vector.tensor_copy(sb[:], ps_blk[:])
                    rec = wp.tile([BLK, H], F32, tag="rec")
                    nc.vector.tensor_scalar_add(rec[:], sb[:, H * F:GVAL], 1e-30)
                    nc.vector.reciprocal(rec[:], rec[:])
                    agg = wp.tile([BLK, F], F32, tag="agg")
                    tmp = wp.tile([BLK, F], F32, tag="tmp")
                    nc.scalar.activation(agg[:], sb[:, 0:F], AF.Identity,
                                         scale=rec[:, 0:1])
                    nc.scalar.activation(tmp[:], sb[:, F:2 * F], AF.Identity,
                                         scale=rec[:, 1:2])
                    nc.vector.tensor_add(out=agg[:], in0=agg[:], in1=tmp[:])
                    agg_bf = wp.tile([BLK, F], BF16, tag="aggbf")
                    nc.vector.tensor_copy(agg_bf[:], agg[:])
                    ps_t = pb.tile([BLK, BLK], BF16, tag="tp", space="PSUM")
                    nc.tensor.transpose(out=ps_t[:F, :], in_=agg_bf[:, :F],
                                        identity=wsb["identb"][:])
                    nc.vector.tensor_add(out=hT_res[:, b0:b0 + bw],
                                         in0=hT_res[:, b0:b0 + bw],
                                         in1=ps_t[:F, :bw])

                # ================= BatchNorm + ReLU ========================
                if stage_cap < 5:
                    continue
                # stats in two block-aligned halves so the first can reduce
                # while phase B still works on the second half's blocks
                half = ASPLIT
                nc.vector.reduce_sum(out=stats[:, 0:1], in_=hT_res[:, 0:half],
                                     axis=mybir.AxisListType.X)
                nc.scalar.square(scr[:, 0:half], hT_res[:, 0:half])
                nc.vector.reduce_sum(out=stats[:, 1:2], in_=scr[:, 0:half],
                                     axis=mybir.AxisListType.X)
                nc.vector.reduce_sum(out=stats[:, 4:5],
                                     in_=hT_res[:, half:NC_N],
                                     axis=mybir.AxisListType.X)
                nc.scalar.square(scr[:, 0:NC_N - half], hT_res[:, half:NC_N])
                nc.vector.reduce_sum(out=stats[:, 5:6], in_=scr[:, 0:NC_N - half],
                                     axis=mybir.AxisListType.X)
                nc.vector.tensor_add(out=stats[:, 0:1], in0=stats[:, 0:1],
                                     in1=stats[:, 4:5])
                nc.vector.tensor_add(out=stats[:, 1:2], in0=stats[:, 1:2],
                                     in1=stats[:, 5:6])
                nc.sync.dma_start(out=bn_src[l][:], in_=stats[:, 0:2])
                nc.gpsimd.collective_compute(
                    "AllReduce", OP.add, replica_groups=groups,
                    ins=[bn_src[l][:]], outs=[bn_out[l][:]])
                nc.sync.dma_start(out=stats[:, 2:4], in_=bn_out[l][:])
                nc.scalar.mul(bnsc[:, 0:1], stats[:, 2:3], 1.0 / N)
                nc.scalar.mul(bnsc[:, 1:2], stats[:, 3:4], 1.0 / N)
                nc.vector.tensor_tensor(out=bnsc[:, 2:3], in0=bnsc[:, 0:1],
                                        in1=bnsc[:, 0:1], op=OP.mult)
                nc.vector.tensor_tensor(out=bnsc[:, 2:3], in0=bnsc[:, 1:2],
                                        in1=bnsc[:, 2:3], op=OP.subtract)
                nc.vector.tensor_scalar_add(bnsc[:, 2:3], bnsc[:, 2:3], BN_EPS)
                nc.vector.reciprocal(bnsc[:, 3:4], bnsc[:, 2:3])
                nc.scalar.sqrt(bnsc[:, 4:5], bnsc[:, 3:4])
                nc.vector.tensor_tensor(out=bnsc[:, 5:6], in0=bnsc[:, 4:5],
                                        in1=wsb[f"bn{l}"][:, 0:1], op=OP.mult)
                nc.vector.tensor_tensor(out=bnsc[:, 6:7], in0=bnsc[:, 0:1],
                                        in1=bnsc[:, 5:6], op=OP.mult)
                nc.vector.tensor_tensor(out=bnsc[:, 6:7], in0=wsb[f"bn{l}"][:, 1:2],
                                        in1=bnsc[:, 6:7], op=OP.subtract)
                # apply per chunk so the next layer / head can start on
                # early chunks while later ones are still being written
                for ci in range(n_chunks):
                    c0 = ci * CHUNK
                    cw = min(CHUNK, NC_N - c0)
                    nc.scalar.activation(hT_act[:, c0:c0 + cw],
                                         hT_res[:, c0:c0 + cw],
                                         AF.Relu, bias=bnsc[:, 6:7],
                                         scale=bnsc[:, 5:6])

            # ================= head ========================================
            for ci in range(n_chunks):
                c0 = ci * CHUNK
                cw = min(CHUNK, NC_N - c0)
                ps_o = pp.tile([3, CHUNK], F32, tag="s", space="PSUM")
                nc.tensor.matmul(out=ps_o[:, :cw], lhsT=wsb["headW"][:],
                                 rhs=hT_act[:F, c0:c0 + cw], start=True, stop=True)
                osb = wp.tile([3, CHUNK], F32, tag="osb")
                nc.scalar.activation(osb[:, :cw], ps_o[:, :cw],
                                     AF.Identity, bias=wsb["headb"][:, 0:1])
                nc.sync.dma_start(out=out_ext[:, c0:c0 + cw], in_=osb[:, :cw])

    return nc


# ---------------------------------------------------------------- run cache
_CACHE = {}


def _build_inputs(inputs, meta, idx_full, dl_dev):
    w = pack_weights(inputs, meta["s_max"])
    x = np.asarray(inputs["x"], np.float32)
    in_maps = []
    for c in range(N_CORES):
        m = dict(w)
        m["xT"] = np.ascontiguousarray(
            x[c * NC_N:(c + 1) * NC_N, :].T).astype(BF)
        m["idx"] = np.ascontiguousarray(idx_full[c])
        m["dstloc"] = np.ascontiguousarray(dl_dev[c])
        in_maps.append(m)
    return in_maps


def kernel(**inputs):
    from concourse.bass_utils import run_bass_kernel_spmd

    _install_hookshim()
    edge = np.asarray(inputs["edge_index"])
    key = hashlib.sha1(edge.tobytes()).hexdigest()
    if key not in _CACHE:
        idx_full, dl_dev, meta = preprocess(edge)
        nc = build_program(meta)
        nc.finalize()
        if os.environ.get("GNN_ASYNC"):
            n_remap, n_del, n_xfer = remap_dmasw_waits(nc)
            print(f"remapped DMASW waits on {n_remap} insts, deleted "
                  f"{n_del} IncSwdgeSem, moved {n_xfer} waits to triggers")
        n_fix = legalize_waits(nc)
        if n_fix:
            print(f"legalize_waits fixed {n_fix} instructions post-finalize")
        _CACHE[key] = (idx_full, dl_dev, meta, nc)
    idx_full, dl_dev, meta, nc = _CACHE[key]
    in_maps = _build_inputs(inputs, meta, idx_full, dl_dev)
    res = run_bass_kernel_spmd(
        nc, in_maps, list(range(N_CORES)),
        trace=bool(os.environ.get("GNN_TRACE")))
    if res.exec_time_ns is not None:
        print(f"HW exec time: {res.exec_time_ns} ns")
    out = np.concatenate([res.results[c]["out"] for c in range(N_CORES)],
                         axis=1)  # [3, N]
    return np.ascontiguousarray(out.T).astype(np.float32)
